# revision 1
# baseline (speedup 1.0000x reference)
"""AttentionConv2D (3x3 windowed multi-head attention) on 8 TRN2 NeuronCores.

Sharding: data-parallel over batch (B=8 -> 1 image per core), weights replicated.
Per-core layout: channel-major [128 ch, 4096 pix]. All cross-channel/window
reductions and broadcasts run on the TensorEngine via block-structured matmuls;
per-pixel products run on DVE/ACT.

Math (host-folded):
  xn = (x - mu)/sqrt(var+eps) * g + b        (LayerNorm over channels)
  z  = (x - mu) * rstd;  xn = z*g + b  ->  fold g into W rows, b into biases.
  Q = z @ Wq' + bq'   (Wq' = diag(g) Wq, bq' = b@Wq + bq); same K', V'.
  scores[p,(n,k)] = (Q[p,n,:] . K[p+dk,n,:] + Q[p,n,:] . pos[k,n,:]) * A^-0.5
  pos-term folds to z @ Wqs + bqs (host-computed).
  attn = softmax_k(scores); out = sum_k attn_k * V_shift_k; final = out @ Wf + bf.

Device pipeline per core (z never materialized; aug rank-1 matmuls add the
-mu*rstd correction and biases):
  xs = x * bcast(rstd)          (bf16)
  Qp = Wq'.T @ xs (+aug)        etc.  -> PSUM, evict to SBUF bf16 (K,V padded)
  P_k = Q . K_pad(shift k)      (DVE bf16)
  scores_psum += BD_k.T @ P_k   (PE, accumulates on pos-scores)
  exp_s = exp(scores)           (ACT) -> SBUF
  denom36 = RS36.T @ exp_s      (PE: sum over k, replicated over k rows)
  attn = exp_s * recip(denom36) (DVE)
  rep_k = E_k.T @ attn          (PE: replicate head scores over 32 o-channels)
  m_k = rep_k * V_pad(shift k)  (DVE/ACT)
  final_psum += Wf.T @ m_k      (PE accumulates the sum over k)
  out = final_psum + bf -> DMA out.
"""

import math
import os
import sys

import numpy as np

sys.path.insert(0, "/opt/trn_rl_repo")

import ml_dtypes  # noqa: E402

BF16 = ml_dtypes.bfloat16

B, CIN, COUT, H, W, KS, NH = 8, 128, 128, 64, 64, 3, 4
A = CIN // NH          # 32
OSH = COUT // NH       # 32
K2 = KS * KS           # 9
NPIX = H * W           # 4096
PW = W + 2             # 66 padded width
PH = H + 2
NPAD = PW * PH + PW + 2  # 4356 + slack so shifted strided views stay in-bounds
NCHUNK = 8
CHUNK = NPIX // NCHUNK  # 512
ROWS_PER_CHUNK = H // NCHUNK  # 8
SCALE = A ** (-0.5)

_CACHE = {}


def _pos_encoding_np():
    pos = np.arange(K2, dtype=np.float32)[:, None]
    div = np.exp(np.arange(0, CIN, 2, dtype=np.float32) * (-math.log(10000.0) / CIN))
    ang = pos * div[None, :]
    return np.stack([np.sin(ang), np.cos(ang)], -1).reshape(K2, CIN)


def _host_fold(ln_g, ln_b, Wq, bq, Wk, bk, Wv, bv, Wp, bp, Wf, bf):
    """All weight-space precomputation (f64 for accuracy, cast at the end)."""
    g = ln_g.astype(np.float64)
    b = ln_b.astype(np.float64)
    Wq = Wq.astype(np.float64); Wk = Wk.astype(np.float64)
    Wv = Wv.astype(np.float64); Wp = Wp.astype(np.float64)
    Wf = Wf.astype(np.float64)
    bq = bq.astype(np.float64); bk = bk.astype(np.float64)
    bv = bv.astype(np.float64); bp = bp.astype(np.float64)
    bfv = bf.astype(np.float64)

    Wq_ = g[:, None] * Wq; bq_ = b @ Wq + bq
    Wk_ = g[:, None] * Wk; bk_ = b @ Wk + bk
    Wv_ = g[:, None] * Wv; bv_ = b @ Wv + bv

    pos = _pos_encoding_np().astype(np.float64) @ Wp + bp  # [K2, NH*A]
    pos = pos.reshape(K2, NH, A)

    # pos-scores: row layout (n,k) = n*9+k ; scores_pos = z @ Wqs + bqs, scaled
    Wqs = np.zeros((CIN, NH * K2))
    bqs = np.zeros((NH * K2,))
    Wq_r = Wq_.reshape(CIN, NH, A)
    bq_r = bq_.reshape(NH, A)
    for n in range(NH):
        for k in range(K2):
            Wqs[:, n * K2 + k] = Wq_r[:, n, :] @ pos[k, n, :]
            bqs[n * K2 + k] = bq_r[n, :] @ pos[k, n, :]
    Wqs *= SCALE
    bqs *= SCALE

    # BD_k [CIN, 36]: (n,a) x (n*9+k) = SCALE ; concat over k -> [128, 9*36]
    bd = np.zeros((K2, CIN, NH * K2))
    for k in range(K2):
        for n in range(NH):
            bd[k, n * A:(n + 1) * A, n * K2 + k] = SCALE
    bd = np.concatenate([bd[k] for k in range(K2)], axis=1)  # [128, 324]

    # E_k [36, 128]: (n*9+k') x (n,o) = 1 iff k'==k ; concat -> [36, 9*128]
    ek = np.zeros((K2, NH * K2, CIN))
    for k in range(K2):
        for n in range(NH):
            ek[k, n * K2 + k, n * OSH:(n + 1) * OSH] = 1.0
    ek = np.concatenate([ek[k] for k in range(K2)], axis=1)  # [36, 1152]

    # RS36 [36, 36]: (n*9+k) x (n'*9+k') = 1 iff n==n'  (sum over k, rep over k')
    rs = np.zeros((NH * K2, NH * K2))
    for n in range(NH):
        rs[n * K2:(n + 1) * K2, n * K2:(n + 1) * K2] = 1.0

    # aug lhsT rows: [colsum(W'); bias] per projection, concat cols: q,k,v,qs
    def aug(Wm, bm):
        return np.stack([Wm.sum(axis=0), bm], axis=0)  # [2, M]

    waug = np.concatenate(
        [aug(Wq_, bq_), aug(Wk_, bk_), aug(Wv_, bv_), aug(Wqs, bqs)], axis=1
    )  # [2, 128*3+36]

    c = {
        "wq": Wq_.astype(BF16), "wk": Wk_.astype(BF16), "wv": Wv_.astype(BF16),
        "wqs": Wqs.astype(BF16), "waug": waug.astype(BF16),
        "bd": bd.astype(BF16), "ek": ek.astype(BF16), "rs": rs.astype(BF16),
        "wf": Wf.astype(BF16),
        "bfb": bfv.astype(np.float32).reshape(COUT, 1),
        "ones_k": np.ones((CIN, 1), dtype=BF16),
        "ones_m": np.ones((1, CIN), dtype=BF16),
        "ones_row": np.ones((1, NPIX), dtype=BF16),
    }
    return c


def _shift_delta(k):
    di, dj = k // KS - 1, k % KS - 1
    return di * PW + dj


def _build_bass():
    import concourse.bass as bass
    import concourse.tile as tile
    from concourse import bacc, mybir

    f32 = mybir.dt.float32
    bf16 = mybir.dt.bfloat16
    AF = mybir.ActivationFunctionType

    nc = bacc.Bacc("TRN2", target_bir_lowering=False, debug=False)

    x_ext = nc.dram_tensor("x", [CIN, NPIX], f32, kind="ExternalInput")
    wq_ext = nc.dram_tensor("wq", [CIN, CIN], bf16, kind="ExternalInput")
    wk_ext = nc.dram_tensor("wk", [CIN, CIN], bf16, kind="ExternalInput")
    wv_ext = nc.dram_tensor("wv", [CIN, CIN], bf16, kind="ExternalInput")
    wqs_ext = nc.dram_tensor("wqs", [CIN, NH * K2], bf16, kind="ExternalInput")
    waug_ext = nc.dram_tensor("waug", [2, 3 * CIN + NH * K2], bf16, kind="ExternalInput")
    bd_ext = nc.dram_tensor("bd", [CIN, K2 * NH * K2], bf16, kind="ExternalInput")
    ek_ext = nc.dram_tensor("ek", [NH * K2, K2 * CIN], bf16, kind="ExternalInput")
    rs_ext = nc.dram_tensor("rs", [NH * K2, NH * K2], bf16, kind="ExternalInput")
    wf_ext = nc.dram_tensor("wf", [COUT, COUT], bf16, kind="ExternalInput")
    bfb_ext = nc.dram_tensor("bfb", [COUT, 1], f32, kind="ExternalInput")
    onek_ext = nc.dram_tensor("ones_k", [CIN, 1], bf16, kind="ExternalInput")
    onem_ext = nc.dram_tensor("ones_m", [1, CIN], bf16, kind="ExternalInput")
    oner_ext = nc.dram_tensor("ones_row", [1, NPIX], bf16, kind="ExternalInput")
    out_ext = nc.dram_tensor("out", [COUT, NPIX], f32, kind="ExternalOutput")

    with tile.TileContext(nc) as tc:
        _kernel_body(tc, nc, mybir, f32, bf16, AF, bass,
                     x_ext, wq_ext, wk_ext, wv_ext, wqs_ext, waug_ext, bd_ext,
                     ek_ext, rs_ext, wf_ext, bfb_ext, onek_ext, onem_ext,
                     oner_ext, out_ext)

    nc.compile()
    return nc


def _kernel_body(tc, nc, mybir, f32, bf16, AF, bass,
                 x_ext, wq_ext, wk_ext, wv_ext, wqs_ext, waug_ext, bd_ext,
                 ek_ext, rs_ext, wf_ext, bfb_ext, onek_ext, onem_ext,
                 oner_ext, out_ext):
    from contextlib import ExitStack

    ctx = ExitStack()
    with ctx:
        consts = ctx.enter_context(tc.tile_pool(name="consts", bufs=1))
        big = ctx.enter_context(tc.tile_pool(name="big", bufs=1))
        mkp = ctx.enter_context(tc.tile_pool(name="mk", bufs=6))
        mallp = ctx.enter_context(tc.tile_pool(name="mall", bufs=3))
        chkp = ctx.enter_context(tc.tile_pool(name="chk", bufs=8))
        smallp = ctx.enter_context(tc.tile_pool(name="small", bufs=1))
        dramp = ctx.enter_context(tc.tile_pool(name="drams", bufs=1, space="DRAM"))
        ps128 = ctx.enter_context(tc.tile_pool(name="ps128", bufs=2, space="PSUM"))
        psacc = ctx.enter_context(tc.tile_pool(name="psacc", bufs=2, space="PSUM"))
        ps36 = ctx.enter_context(tc.tile_pool(name="ps36", bufs=4, space="PSUM"))

        def mm(out, lhsT, rhs, **kw):
            nc.tensor.matmul(out, lhsT, rhs, **kw)

        # ---- load constants ----
        wq = consts.tile([CIN, CIN], bf16); nc.sync.dma_start(wq[:], wq_ext[:])
        wk = consts.tile([CIN, CIN], bf16); nc.sync.dma_start(wk[:], wk_ext[:])
        wv = consts.tile([CIN, CIN], bf16); nc.sync.dma_start(wv[:], wv_ext[:])
        wqs = consts.tile([CIN, NH * K2], bf16); nc.sync.dma_start(wqs[:], wqs_ext[:])
        waug = consts.tile([2, 3 * CIN + NH * K2], bf16)
        nc.sync.dma_start(waug[:], waug_ext[:])
        bdw = consts.tile([CIN, K2 * NH * K2], bf16); nc.sync.dma_start(bdw[:], bd_ext[:])
        ekw = consts.tile([NH * K2, K2 * CIN], bf16); nc.sync.dma_start(ekw[:], ek_ext[:])
        rsw = consts.tile([NH * K2, NH * K2], bf16); nc.sync.dma_start(rsw[:], rs_ext[:])
        wf = consts.tile([COUT, COUT], bf16); nc.sync.dma_start(wf[:], wf_ext[:])
        bfb = consts.tile([COUT, 1], f32); nc.sync.dma_start(bfb[:], bfb_ext[:])
        ones_k = consts.tile([CIN, 1], bf16); nc.sync.dma_start(ones_k[:], onek_ext[:])
        ones_m = consts.tile([1, CIN], bf16); nc.sync.dma_start(ones_m[:], onem_ext[:])

        # ---- input image ----
        x_sb = big.tile([CIN, NPIX], f32)
        QT = NPIX // 4
        for _i in range(4):
            nc.sync.dma_start(x_sb[:, _i * QT:(_i + 1) * QT],
                              x_ext[:, _i * QT:(_i + 1) * QT])

        # ---- stats: S1 = sum_c x, S2 = sum_c x^2 (bf16 inputs, fp32 psum) ----
        s1_rows = smallp.tile([1, NPIX], f32, tag="s1_rows")
        s2_rows = smallp.tile([1, NPIX], f32, tag="s2_rows")
        for c in range(NCHUNK):
            sl = slice(c * CHUNK, (c + 1) * CHUNK)
            x_bf = mkp.tile([CIN, CHUNK], bf16, tag="xbf")
            nc.gpsimd.tensor_copy(x_bf[:], x_sb[:, sl])
            sq_bf = mkp.tile([CIN, CHUNK], bf16, tag="sqbf")
            nc.gpsimd.tensor_tensor(sq_bf[:], x_sb[:, sl], x_sb[:, sl],
                                    mybir.AluOpType.mult)
            s1 = ps36.tile([1, CHUNK], f32, tag="ps36")
            mm(s1[:], ones_k[:], x_bf[:], start=True, stop=True)
            s2 = ps36.tile([1, CHUNK], f32, tag="ps36")
            mm(s2[:], ones_k[:], sq_bf[:], start=True, stop=True)
            nc.vector.tensor_copy(s1_rows[0:1, sl], s1[:])
            nc.scalar.copy(s2_rows[0:1, sl], s2[:])

        # ---- pack stats via DRAM bounce -> [128, 64] ----
        s_dram = dramp.tile([2, NPIX], f32)
        nc.sync.dma_start(s_dram[0:1, :], s1_rows[:])
        nc.sync.dma_start(s_dram[1:2, :], s2_rows[:])
        s_pack = smallp.tile([CIN, 2 * NPIX // CIN], f32, tag="s_pack")  # [128, 64]
        PCK = NPIX // CIN  # 32
        nc.sync.dma_start(s_pack[:, 0:PCK], s_dram[0, :].rearrange("(p j) -> p j", p=CIN))
        nc.sync.dma_start(s_pack[:, PCK:2 * PCK], s_dram[1, :].rearrange("(p j) -> p j", p=CIN))

        # rstd = 1/sqrt(S2/128 - (S1/128)^2 + eps); sneg = -(S1/128)*rstd
        S1 = s_pack[:, 0:PCK]
        S2 = s_pack[:, PCK:2 * PCK]
        stat2 = smallp.tile([CIN, 4 * PCK], f32, tag="stat2")
        mean = stat2[:, 0:PCK]
        var = stat2[:, PCK:2 * PCK]
        rstd = stat2[:, 2 * PCK:3 * PCK]
        sneg = stat2[:, 3 * PCK:4 * PCK]
        nc.vector.tensor_scalar_mul(mean[:], S1[:], 1.0 / CIN)
        nc.vector.tensor_tensor(var[:], mean[:], mean[:], mybir.AluOpType.mult)
        nc.vector.tensor_scalar_mul(S2[:], S2[:], 1.0 / CIN)
        nc.vector.tensor_tensor(var[:], S2[:], var[:], mybir.AluOpType.subtract)
        nc.vector.tensor_scalar_add(var[:], var[:], 1e-5)
        nc.scalar.sqrt(var[:], var[:])              # std
        nc.vector.reciprocal_approx_fast(rstd[:], var[:])
        nc.vector.tensor_tensor(sneg[:], mean[:], rstd[:], mybir.AluOpType.mult)
        nc.vector.tensor_scalar_mul(sneg[:], sneg[:], -1.0)
        stat_bf = smallp.tile([CIN, 2 * PCK], bf16, tag="stat_bf")
        nc.vector.tensor_copy(stat_bf[:, 0:PCK], rstd[:])
        nc.vector.tensor_copy(stat_bf[:, PCK:2 * PCK], sneg[:])

        # unpack to rows via DRAM bounce
        r_dram = dramp.tile([2 * PCK * CIN], bf16)
        nc.sync.dma_start(r_dram[0:PCK * CIN].rearrange("(p j) -> p j", p=CIN),
                          stat_bf[:, 0:PCK])
        nc.sync.dma_start(r_dram[PCK * CIN:].rearrange("(p j) -> p j", p=CIN),
                          stat_bf[:, PCK:2 * PCK])
        rstd_row = smallp.tile([1, NPIX], bf16, tag="rstd_row")
        nc.sync.dma_start(rstd_row[:], r_dram[0:NPIX].rearrange("(o p) -> o p", o=1))
        srow2 = smallp.tile([2, NPIX], bf16, tag="srow2")
        nc.sync.dma_start(srow2[0:1, :], r_dram[NPIX:].rearrange("(o p) -> o p", o=1))
        nc.sync.dma_start(srow2[1:2, :], oner_ext[:])

        # ---- padded K/V buffers (zero borders) ----
        k_pad = big.tile([CIN, NPAD], bf16)
        v_pad = big.tile([CIN, NPAD], bf16)
        nc.gpsimd.memset(k_pad[:], 0.0)
        nc.gpsimd.memset(v_pad[:], 0.0)
        q_tiles = [None] * NCHUNK
        xs_tiles = [None] * NCHUNK

        def pad_view(t, c, delta=0):
            # rows c*8 .. c*8+7 of padded buffer, inner 64 cols, shifted by delta
            off = (1 + c * ROWS_PER_CHUNK) * PW + 1 + delta
            return t[:, off:off + ROWS_PER_CHUNK * PW].rearrange(
                "p (r w) -> p r w", r=ROWS_PER_CHUNK, w=PW)[:, :, 0:W]

        AUGQ, AUGK, AUGV, AUGS = (slice(0, CIN), slice(CIN, 2 * CIN),
                                  slice(2 * CIN, 3 * CIN),
                                  slice(3 * CIN, 3 * CIN + NH * K2))

        # ---- phase 2: normalize + projections (fills q_sb, k_pad, v_pad) ----
        for c in range(NCHUNK):
            sl = slice(c * CHUNK, (c + 1) * CHUNK)
            rb = ps128.tile([CIN, CHUNK], f32, tag="ps128")
            mm(rb[:], ones_m[:], rstd_row[:, sl], start=True, stop=True)
            xs_c = chkp.tile([CIN, CHUNK], bf16, tag="xs")
            xs_tiles[c] = xs_c
            nc.vector.tensor_tensor(xs_c[:], x_sb[:, sl], rb[:], mybir.AluOpType.mult)

            qp = ps128.tile([CIN, CHUNK], f32, tag="ps128")
            mm(qp[:], wq[:], xs_c[:], start=True, stop=False)
            mm(qp[:], waug[:, AUGQ], srow2[:, sl], start=False, stop=True)
            q_c = chkp.tile([CIN, CHUNK], bf16, tag="q")
            q_tiles[c] = q_c
            nc.vector.tensor_copy(q_c[:], qp[:])

            kp = ps128.tile([CIN, CHUNK], f32, tag="ps128")
            mm(kp[:], wk[:], xs_c[:], start=True, stop=False)
            mm(kp[:], waug[:, AUGK], srow2[:, sl], start=False, stop=True)
            nc.vector.tensor_copy(pad_view(k_pad, c)[:], kp[:].rearrange(
                "p (r w) -> p r w", r=ROWS_PER_CHUNK, w=W))

            vp = ps128.tile([CIN, CHUNK], f32, tag="ps128")
            mm(vp[:], wv[:], xs_c[:], start=True, stop=False)
            mm(vp[:], waug[:, AUGV], srow2[:, sl], start=False, stop=True)
            nc.scalar.copy(pad_view(v_pad, c)[:], vp[:].rearrange(
                "p (r w) -> p r w", r=ROWS_PER_CHUNK, w=W))

        # ---- phase 3+4 per chunk: scores, softmax, AV, Wf, out ----
        for c in range(NCHUNK):
            sl = slice(c * CHUNK, (c + 1) * CHUNK)
            q_v = q_tiles[c][:].rearrange("p (r w) -> p r w", r=ROWS_PER_CHUNK, w=W)

            sc = ps36.tile([NH * K2, CHUNK], f32, tag="ps36")
            mm(sc[:], wqs[:], xs_tiles[c][:], start=True, stop=False)
            mm(sc[:], waug[:, AUGS], srow2[:, sl], start=False, stop=False)
            for k in range(K2):
                pk = mkp.tile([CIN, CHUNK], bf16, tag="pk")
                pk_v = pk[:].rearrange("p (r w) -> p r w", r=ROWS_PER_CHUNK, w=W)
                eng = nc.gpsimd if k in (0, 5) else nc.vector
                eng.tensor_tensor(pk_v[:], q_v[:],
                                  pad_view(k_pad, c, _shift_delta(k))[:],
                                  mybir.AluOpType.mult)
                mm(sc[:], bdw[:, k * NH * K2:(k + 1) * NH * K2], pk[:],
                   start=False, stop=(k == K2 - 1))

            exp_c = chkp.tile([NH * K2, CHUNK], bf16, tag="exp")
            nc.scalar.activation(exp_c[:], sc[:], AF.Exp)
            dn = ps36.tile([NH * K2, CHUNK], f32, tag="ps36")
            mm(dn[:], rsw[:], exp_c[:], start=True, stop=True)
            rcp36 = mkp.tile([NH * K2, CHUNK], f32, tag="rcp")
            nc.vector.reciprocal_approx_fast(rcp36[:], dn[:])
            attn_c = chkp.tile([NH * K2, CHUNK], bf16, tag="attn")
            nc.vector.tensor_tensor(attn_c[:], exp_c[:], rcp36[:],
                                    mybir.AluOpType.mult)

            acc = psacc.tile([COUT, CHUNK], f32, tag="acc")
            m_all = mallp.tile([CIN, K2 * CHUNK], bf16, tag="mall")
            for k in range(K2):
                rep = ps128.tile([CIN, CHUNK], f32, tag="ps128")
                mm(rep[:], ekw[:, k * CIN:(k + 1) * CIN], attn_c[:],
                   start=True, stop=True)
                rep_sb = mkp.tile([CIN, CHUNK], bf16, tag="repsb")
                nc.scalar.copy(rep_sb[:], rep[:])
                mk_v = m_all[:, k * CHUNK:(k + 1) * CHUNK].rearrange(
                    "p (r w) -> p r w", r=ROWS_PER_CHUNK, w=W)
                nc.vector.tensor_tensor(
                    mk_v[:], rep_sb[:].rearrange("p (r w) -> p r w",
                                                 r=ROWS_PER_CHUNK, w=W),
                    pad_view(v_pad, c, _shift_delta(k))[:],
                    mybir.AluOpType.mult)
                mm(acc[:], wf[:], m_all[:, k * CHUNK:(k + 1) * CHUNK],
                   start=(k == 0), stop=(k == K2 - 1))
            out_sb = mkp.tile([COUT, CHUNK], f32, tag="outsb")
            nc.vector.tensor_scalar_add(out_sb[:], acc[:], bfb[:])
            nc.sync.dma_start(out_ext[:, sl], out_sb[:])


def _get_compiled():
    if "nc" not in _CACHE:
        _CACHE["nc"] = _build_bass()
    return _CACHE["nc"]


def kernel(**inputs):
    x = np.asarray(inputs["x"], dtype=np.float32)          # [B, CIN, H, W]
    consts = _host_fold(
        np.asarray(inputs["ln_g"]), np.asarray(inputs["ln_b"]),
        np.asarray(inputs["Wq"]), np.asarray(inputs["bq"]),
        np.asarray(inputs["Wk"]), np.asarray(inputs["bk"]),
        np.asarray(inputs["Wv"]), np.asarray(inputs["bv"]),
        np.asarray(inputs["Wp"]), np.asarray(inputs["bp"]),
        np.asarray(inputs["Wf"]), np.asarray(inputs["bf"]),
    )

    nc = _get_compiled()

    from concourse.bass_utils import run_bass_kernel_spmd

    core_ids = list(range(B))
    in_maps = []
    for i in range(B):
        m = {"x": np.ascontiguousarray(x[i].reshape(CIN, NPIX))}
        m.update(consts)
        in_maps.append(m)

    res = run_bass_kernel_spmd(nc, in_maps, core_ids,
                               trace=bool(int(os.environ.get("KTRACE", "0"))))
    _CACHE["last_result"] = res
    out = np.stack([res.results[i]["out"].reshape(COUT, H, W) for i in range(B)])
    return out.astype(np.float32)


if __name__ == "__main__":
    # smoke build only
    nc = _get_compiled()
    print("compiled OK")



# revision 5
# speedup vs baseline: 1.1882x; 1.1882x over previous
"""AttentionConv2D (3x3 windowed multi-head attention) on 8 TRN2 NeuronCores. v2.

Sharding: data-parallel over batch (B=8 -> 1 image per core), weights replicated.
Per-core layout: channel-major [128 ch, 4096 pix].

v2 vs v1: no aug matmuls (pre-centered z via DMA partition-broadcast of mu/rstd
rows), biases folded into ACT evictions / Exp bias, SBUF-SBUF stats
redistribution, engine-balanced elementwise work, software-pipelined chunks.
"""

import math
import os
import sys

import numpy as np

sys.path.insert(0, "/opt/trn_rl_repo")

import ml_dtypes  # noqa: E402

BF16 = ml_dtypes.bfloat16

B, CIN, COUT, H, W, KS, NH = 8, 128, 128, 64, 64, 3, 4
A = CIN // NH          # 32
OSH = COUT // NH       # 32
K2 = KS * KS           # 9
NPIX = H * W           # 4096
PW = W + 2             # 66 padded width
PH = H + 2
NPAD = PW * PH + PW + 2  # slack so shifted strided views stay in-bounds
NCHUNK = 8
CHUNK = NPIX // NCHUNK  # 512
ROWS_PER_CHUNK = H // NCHUNK  # 8
SCALE = A ** (-0.5)
PCK = NPIX // CIN      # 32 packed-stat columns per row

_CACHE = {}


def _pos_encoding_np():
    pos = np.arange(K2, dtype=np.float32)[:, None]
    div = np.exp(np.arange(0, CIN, 2, dtype=np.float32) * (-math.log(10000.0) / CIN))
    ang = pos * div[None, :]
    return np.stack([np.sin(ang), np.cos(ang)], -1).reshape(K2, CIN)


def _host_fold(ln_g, ln_b, Wq, bq, Wk, bk, Wv, bv, Wp, bp, Wf, bf):
    """All weight-space precomputation (f64 for accuracy, cast at the end)."""
    g = ln_g.astype(np.float64)
    b = ln_b.astype(np.float64)
    Wq = Wq.astype(np.float64); Wk = Wk.astype(np.float64)
    Wv = Wv.astype(np.float64); Wp = Wp.astype(np.float64)
    Wf = Wf.astype(np.float64)
    bq = bq.astype(np.float64); bk = bk.astype(np.float64)
    bv = bv.astype(np.float64); bp = bp.astype(np.float64)
    bfv = bf.astype(np.float64)

    Wq_ = g[:, None] * Wq; bq_ = b @ Wq + bq
    Wk_ = g[:, None] * Wk; bk_ = b @ Wk + bk
    Wv_ = g[:, None] * Wv; bv_ = b @ Wv + bv

    pos = _pos_encoding_np().astype(np.float64) @ Wp + bp  # [K2, NH*A]
    pos = pos.reshape(K2, NH, A)

    # pos-scores: row layout (n,k) = n*9+k ; scores_pos = z @ Wqs + bqs, scaled
    Wqs = np.zeros((CIN, NH * K2))
    bqs = np.zeros((NH * K2,))
    Wq_r = Wq_.reshape(CIN, NH, A)
    bq_r = bq_.reshape(NH, A)
    for n in range(NH):
        for k in range(K2):
            Wqs[:, n * K2 + k] = Wq_r[:, n, :] @ pos[k, n, :]
            bqs[n * K2 + k] = bq_r[n, :] @ pos[k, n, :]
    Wqs *= SCALE
    bqs *= SCALE

    # BD_k [CIN, 36]: (n,a) x (n*9+k) = SCALE ; concat over k -> [128, 9*36]
    bd = np.zeros((K2, CIN, NH * K2))
    for k in range(K2):
        for n in range(NH):
            bd[k, n * A:(n + 1) * A, n * K2 + k] = SCALE
    bd = np.concatenate([bd[k] for k in range(K2)], axis=1)  # [128, 324]

    # E_k [36, 128]: (n*9+k') x (n,o) = 1 iff k'==k ; concat -> [36, 9*128]
    ek = np.zeros((K2, NH * K2, CIN))
    for k in range(K2):
        for n in range(NH):
            ek[k, n * K2 + k, n * OSH:(n + 1) * OSH] = 1.0
    ek = np.concatenate([ek[k] for k in range(K2)], axis=1)  # [36, 1152]

    # RS36 [36, 36]: (n*9+k) x (n'*9+k') = 1 iff n==n'  (sum over k, rep over k')
    rs = np.zeros((NH * K2, NH * K2))
    for n in range(NH):
        rs[n * K2:(n + 1) * K2, n * K2:(n + 1) * K2] = 1.0

    def pad128(m):
        out = np.zeros((CIN, m.shape[1]))
        out[:m.shape[0]] = m
        return out

    # one concatenated bf16 const blob [128, 2061]:
    # wq(0:128) wk(128:256) wv(256:384) wqs(384:420) bd(420:744) wf(744:872)
    # ones(872:873) ek(873:2025) rs(2025:2061)
    cb16 = np.concatenate([
        Wq_, Wk_, Wv_, Wqs, bd, Wf, np.ones((CIN, 1)), pad128(ek), pad128(rs),
        np.eye(CIN), np.ones((CIN, CIN)),
    ], axis=1).astype(BF16)
    # f32 bias blob [128, 5]: bqc bkc bvc bfb bqsc(pad)
    bfv2 = bfv + (1.0 + K2 * 1e-8) * (bv_ @ Wf)
    cf32 = np.stack([
        bq_, bk_, bv_, bfv2, np.concatenate([bqs, np.zeros(CIN - NH * K2)]),
        np.ones(CIN),
    ], axis=1).astype(np.float32)
    return {"cb16": np.ascontiguousarray(cb16), "cf32": np.ascontiguousarray(cf32)}


def _shift_delta(k):
    di, dj = k // KS - 1, k % KS - 1
    return di * PW + dj


def _build_bass():
    import concourse.bass as bass
    import concourse.tile as tile
    from concourse import bacc, mybir

    f32 = mybir.dt.float32
    bf16 = mybir.dt.bfloat16
    AF = mybir.ActivationFunctionType

    nc = bacc.Bacc("TRN2", target_bir_lowering=False, debug=False)

    ext = {}
    ext["x"] = nc.dram_tensor("x", [CIN, NPIX], f32, kind="ExternalInput")
    ext["cb16"] = nc.dram_tensor("cb16", [CIN, 2317], bf16, kind="ExternalInput")
    ext["cf32"] = nc.dram_tensor("cf32", [CIN, 6], f32, kind="ExternalInput")
    out_ext = nc.dram_tensor("out", [COUT, NPIX], f32, kind="ExternalOutput")

    with tile.TileContext(nc) as tc:
        _kernel_body(tc, nc, mybir, f32, bf16, AF, bass, ext, out_ext)

    nc.compile()
    return nc


def _kernel_body(tc, nc, mybir, f32, bf16, AF, bass, ext, out_ext):
    from contextlib import ExitStack

    f32r = mybir.dt.float32r
    mult = mybir.AluOpType.mult
    sub = mybir.AluOpType.subtract

    ctx = ExitStack()
    with ctx:
        consts = ctx.enter_context(tc.tile_pool(name="consts", bufs=1))
        big = ctx.enter_context(tc.tile_pool(name="big", bufs=1))
        xbfp = ctx.enter_context(tc.tile_pool(name="xbf", bufs=2))
        sqp = ctx.enter_context(tc.tile_pool(name="sqp", bufs=2))
        tmpp = ctx.enter_context(tc.tile_pool(name="tmpp", bufs=3))
        zp = ctx.enter_context(tc.tile_pool(name="zp", bufs=4))
        qp_pool = ctx.enter_context(tc.tile_pool(name="qpool", bufs=4))
        pkp = ctx.enter_context(tc.tile_pool(name="pkp", bufs=6))
        mkp = ctx.enter_context(tc.tile_pool(name="mkp", bufs=6))
        repp = ctx.enter_context(tc.tile_pool(name="repp", bufs=6))
        smallp = ctx.enter_context(tc.tile_pool(name="small", bufs=3))
        statp = ctx.enter_context(tc.tile_pool(name="statp", bufs=1))
        dramp = ctx.enter_context(tc.tile_pool(name="drams", bufs=1, space="DRAM"))
        outp = ctx.enter_context(tc.tile_pool(name="outp", bufs=2))
        ps_a = ctx.enter_context(tc.tile_pool(name="ps_a", bufs=1, space="PSUM"))
        ps_s = ctx.enter_context(tc.tile_pool(name="ps_s", bufs=2, space="PSUM"))
        ps_r = ctx.enter_context(tc.tile_pool(name="ps_r", bufs=3, space="PSUM"))
        ps_o = ctx.enter_context(tc.tile_pool(name="ps_o", bufs=2, space="PSUM"))

        def mm(out, lhsT, rhs, **kw):
            nc.tensor.matmul(out, lhsT, rhs, **kw)

        # ---- big SBUF buffers ----
        x_sb = big.tile([CIN, NPIX], f32)
        k_pad = big.tile([CIN, NPAD], bf16)
        v_pad = big.tile([CIN, NPAD], bf16)
        smb = big.tile([CIN, 2 * NPIX], bf16)  # [rstd | mu] broadcast cols

        # ---- preload ACT tables with dummy ops on a zeroed scratch ----
        scr = statp.tile([1, 4], f32, tag="scr")
        nc.vector.memset(scr[:], 1.0)
        nc.scalar.square(scr[:, 1:2], scr[:, 0:1])
        nc.scalar.sqrt(scr[:, 2:3], scr[:, 0:1])
        nc.scalar.activation(scr[:, 3:4], scr[:, 0:1], AF.Exp)
        nc.scalar.copy(scr[:, 1:2], scr[:, 0:1])
        nc.scalar.add(scr[:, 2:3], scr[:, 0:1], scr[:, 0:1])

        # ---- input + constants (x quarter 0 first, then consts) ----
        nc.scalar.dma_start(out=x_sb[:, 0:CHUNK], in_=ext["x"][:, 0:CHUNK])
        nc.scalar.dma_start(out=x_sb[:, CHUNK:NPIX // 4],
                            in_=ext["x"][:, CHUNK:NPIX // 4])
        cb16 = consts.tile([CIN, 2317], bf16)
        nc.sync.dma_start(cb16[:], ext["cb16"][:])
        cf32 = consts.tile([CIN, 6], f32)
        nc.sync.dma_start(cf32[:], ext["cf32"][:])
        for qx in range(1, 4):
            sl = slice(qx * NPIX // 4, (qx + 1) * NPIX // 4)
            nc.scalar.dma_start(out=x_sb[:, sl], in_=ext["x"][:, sl])
        wq = cb16[:, 0:128]
        wk = cb16[:, 128:256]
        wv = cb16[:, 256:384]
        wqs = cb16[:, 384:420]
        bdw = cb16[:, 420:744]
        wf = cb16[:, 744:872]
        ones_k = cb16[:, 872:873]
        ekw = cb16[0:NH * K2, 873:2025]
        rsw = cb16[0:NH * K2, 2025:2061]
        ident = cb16[:, 2061:2189]
        ones_row = cb16[0:1, 2189:2317]
        ident16 = cb16[0:1, 2061:2062]
        bqc = cf32[:, 0:1]
        bkc = cf32[:, 1:2]
        bvc = cf32[:, 2:3]
        bfb = cf32[:, 3:4]
        bqsc = cf32[0:NH * K2, 4:5]
        one32 = cf32[0:1, 5:6]
        ones32r = cf32[:, 5:6].bitcast(mybir.dt.float32r)

        # ---- stats, issued per quarter so LN finalize overlaps later chunks ----
        # s12row: single row, s1 at [0, j], s2 at [0, NPIX + j]
        s12row = statp.tile([1, 2 * NPIX], f32, tag="s12row")
        s_dram = dramp.tile([2, NPIX], bf16)
        QPIX = NPIX // 4      # 1024 pixels per quarter
        QCK = QPIX // CIN     # 8 packed columns per quarter

        def stats_chunk(c):
            sl = slice(c * CHUNK, (c + 1) * CHUNK)
            x_bf = xbfp.tile([CIN, CHUNK], bf16, tag="xbf")
            nc.gpsimd.tensor_copy(x_bf[:], x_sb[:, sl])            # Pool
            yield
            sq_bf = sqp.tile([CIN, CHUNK], bf16, tag="sq")
            nc.scalar.square(sq_bf[:], x_bf[:])                    # ACT
            yield
            s1 = ps_s.tile([1, CHUNK], f32, tag="pss")
            mm(s1[:], ones_k, x_bf[:], start=True, stop=True)
            yield
            s2 = ps_s.tile([1, CHUNK], f32, tag="pss")
            mm(s2[:], ones_k, sq_bf[:], start=True, stop=True)
            yield
            nc.vector.tensor_copy(s12row[0:1, sl], s1[:])          # DVE evict
            yield
            s2dst = s12row[0:1, NPIX + c * CHUNK:NPIX + (c + 1) * CHUNK]
            if c % 2 == 0:
                nc.scalar.copy(s2dst, s2[:])                       # ACT evict
            else:
                nc.vector.tensor_copy(s2dst, s2[:])                # DVE evict
            yield

        def stats_quarter(qr):
            yield from stats_chunk(2 * qr)
            yield from stats_chunk(2 * qr + 1)
            yield from stats_finalize(qr)

        def fin_pe(g):
            # PE-path LN finalize for head chunk g (pixels g*512..g*512+511):
            # pack via transposes, math, transpose rows back, bcast matmuls
            # into PSUM (rbps/mbps) read directly by the centering ops.
            GC = 4  # 512 px / 128
            base = g * CHUNK
            tps = ps_o.tile([CIN, 2 * GC], f32, tag="acc")
            for j in range(GC):
                o1 = base + j * CIN
                nc.tensor.transpose(tps[:, j:j + 1],
                                    s12row[0:1, o1:o1 + CIN], one32)
                o2 = NPIX + base + j * CIN
                nc.tensor.transpose(tps[:, GC + j:GC + j + 1],
                                    s12row[0:1, o2:o2 + CIN], one32)
            yield
            S1 = tps[:, 0:GC]
            S2 = tps[:, GC:2 * GC]
            stat2 = statp.tile([CIN, 3 * GC], f32, tag=f"fpe{g}")
            mean = stat2[:, 0:GC]
            msq = stat2[:, GC:2 * GC]
            var = stat2[:, 2 * GC:3 * GC]
            nc.vector.tensor_scalar_mul(mean[:], S1[:], 1.0 / CIN)
            yield
            nc.vector.tensor_tensor(msq[:], mean[:], mean[:], mult)
            nc.vector.scalar_tensor_tensor(var[:], S2[:], 1.0 / CIN, msq[:],
                                           mult, sub)
            nc.vector.tensor_scalar_add(var[:], var[:], 1e-5)
            yield
            stdg = statp.tile([CIN, GC], f32, tag=f"fpestd{g}")
            nc.scalar.sqrt(stdg[:], var[:])
            yield
            rstdg = statp.tile([CIN, GC], f32, tag=f"fper{g}")
            nc.vector.reciprocal_approx_fast(rstdg[:], stdg[:])
            yield
            sbfg = statp.tile([CIN, 2 * GC], bf16, tag=f"fpeb{g}")
            nc.vector.tensor_copy(sbfg[:, 0:GC], rstdg[:])
            nc.vector.tensor_copy(sbfg[:, GC:2 * GC], mean[:])
            yield
            # rows: T[j,p]: j 0-3 rstd segments, 4-7 mean segments
            tr = ps_o.tile([2 * GC, CIN], bf16, tag="acc")
            nc.tensor.transpose(tr[:], sbfg[:], ident)
            yield
            srow8 = statp.tile([1, 2 * GC * CIN], bf16, tag=f"fpes{g}")
            engs = [nc.scalar, nc.vector, nc.gpsimd]
            for j in range(2 * GC):
                eng = engs[j % 3]
                if eng is nc.vector:
                    eng.tensor_copy(srow8[0:1, j * CIN:(j + 1) * CIN],
                                    tr[j:j + 1, :])
                elif eng is nc.gpsimd:
                    eng.tensor_copy(srow8[0:1, j * CIN:(j + 1) * CIN],
                                    tr[j:j + 1, :])
                else:
                    eng.copy(srow8[0:1, j * CIN:(j + 1) * CIN], tr[j:j + 1, :])
            yield
            rbp = ps_r.tile([CIN, CHUNK], f32, tag="rep")
            mbp = ps_r.tile([CIN, CHUNK], f32, tag="rep")
            rbps[g], mbps[g] = rbp, mbp
            for j in range(GC):
                mm(rbp[:, j * CIN:(j + 1) * CIN], ones_row,
                   srow8[0:1, j * CIN:(j + 1) * CIN], start=True, stop=True)
                mm(mbp[:, j * CIN:(j + 1) * CIN], ones_row,
                   srow8[0:1, (GC + j) * CIN:(GC + j + 1) * CIN],
                   start=True, stop=True)
            yield

        def transpose_pack(qr, tps, half):
            for j in range(half * QCK // 2, (half + 1) * QCK // 2):
                o1 = qr * QPIX + j * CIN
                nc.tensor.transpose(tps[:, j:j + 1],
                                    s12row[0:1, o1:o1 + CIN], one32)
                o2 = NPIX + qr * QPIX + j * CIN
                nc.tensor.transpose(tps[:, QCK + j:QCK + j + 1],
                                    s12row[0:1, o2:o2 + CIN], one32)

        def stats_finalize(qr):
            qsl = slice(qr * QPIX, (qr + 1) * QPIX)
            qsl2 = slice(NPIX + qr * QPIX, NPIX + (qr + 1) * QPIX)
            # pack quarter via PE transposes: tps[p, b*QCK+j] = s_b[qr*1024+j*128+p]
            tps = ps_s.tile([CIN, 2 * QCK], f32, tag="pss")
            transpose_pack(qr, tps, 0)
            yield
            transpose_pack(qr, tps, 1)
            yield
            S1 = tps[:, 0:QCK]
            S2 = tps[:, QCK:2 * QCK]
            stat2 = statp.tile([CIN, 3 * QCK], f32, tag=f"stat2{qr}")
            mean = stat2[:, 0:QCK]
            msq = stat2[:, QCK:2 * QCK]
            var = stat2[:, 2 * QCK:3 * QCK]
            nc.vector.tensor_scalar_mul(mean[:], S1[:], 1.0 / CIN)
            yield
            nc.vector.tensor_tensor(msq[:], mean[:], mean[:], mult)
            nc.vector.scalar_tensor_tensor(var[:], S2[:], 1.0 / CIN, msq[:], mult, sub)
            nc.vector.tensor_scalar_add(var[:], var[:], 1e-5)
            yield
            std = statp.tile([CIN, QCK], f32, tag=f"std{qr}")
            nc.scalar.sqrt(std[:], var[:])
            rstd32 = statp.tile([CIN, QCK], f32, tag=f"rstd32{qr}")
            nc.vector.reciprocal_approx_fast(rstd32[:], std[:])
            stat_bf = statp.tile([CIN, 2 * QCK], bf16, tag=f"stat_bf{qr}")
            nc.vector.tensor_copy(stat_bf[:, 0:QCK], rstd32[:])
            yield
            nc.vector.tensor_copy(stat_bf[:, QCK:2 * QCK], mean[:])
            yield
            # DMAs to DRAM rows; pixel index = qr*1024 + j*128 + p
            dd0 = s_dram[0:1, 0:1]
            for row, scols in ((0, slice(0, QCK)), (1, slice(QCK, 2 * QCK))):
                ddst = bass.AP(tensor=dd0.tensor,
                               offset=dd0.offset + row * NPIX + qr * QPIX,
                               ap=[[1, CIN], [CIN, QCK]])
                nc.sync.dma_start(ddst, stat_bf[:, scols])
                yield
            # partition-broadcast back into smb ([rstd | mu] column blocks)
            dd = s_dram[0:1, 0:1]
            for row, dcols in ((0, qsl), (1, qsl2)):
                src = bass.AP(tensor=dd.tensor,
                              offset=dd.offset + row * NPIX + qr * QPIX,
                              ap=[[0, CIN], [1, QPIX]])
                nc.sync.dma_start(smb[:, dcols], src)
                yield

        for pad_t in (k_pad, v_pad):
            nc.gpsimd.memset(pad_t[:, 0:PW + 1], 0.0)
            nc.gpsimd.memset(
                pad_t[:, PW + 65:PW + 65 + 64 * PW].rearrange(
                    "p (r t) -> p r t", t=PW)[:, :, 0:2], 0.0)
            nc.gpsimd.memset(pad_t[:, 65 * PW + 1:NPAD], 0.0)

        z_tiles = [None] * NCHUNK
        q_tiles = [None] * NCHUNK

        def pad_view(t, c, delta=0):
            off = (1 + c * ROWS_PER_CHUNK) * PW + 1 + delta
            return t[:, off:off + ROWS_PER_CHUNK * PW].rearrange(
                "p (r w) -> p r w", r=ROWS_PER_CHUNK, w=PW)[:, :, 0:W]

        def proj_gen(c):
            sl = slice(c * CHUNK, (c + 1) * CHUNK)
            tmp = tmpp.tile([CIN, CHUNK], bf16, tag="tmp")
            nc.vector.tensor_tensor(
                tmp[:], x_sb[:, sl],
                smb[:, NPIX + c * CHUNK:NPIX + (c + 1) * CHUNK], sub)
            yield
            z = zp.tile([CIN, CHUNK], bf16, tag="z")
            z_tiles[c] = z
            nc.vector.tensor_tensor(z[:], tmp[:], smb[:, sl], mult)
            yield
            qps = ps_a.tile([CIN, CHUNK], f32, tag="ps_a")
            mm(qps[:], wq, z[:], start=True, stop=True)
            yield
            q_c = qp_pool.tile([CIN, CHUNK], bf16, tag="q")
            q_tiles[c] = q_c
            nc.scalar.add(q_c[:], qps[:], bqc)                  # ACT
            yield
            kps = ps_a.tile([CIN, CHUNK], f32, tag="ps_a")
            mm(kps[:], wk, z[:], start=True, stop=True)
            yield
            nc.scalar.add(pad_view(k_pad, c)[:],
                          kps[:].rearrange("p (r w) -> p r w",
                                           r=ROWS_PER_CHUNK, w=W), bkc)  # ACT
            yield
            vps = ps_a.tile([CIN, CHUNK], f32, tag="ps_a")
            mm(vps[:], wv, z[:], start=True, stop=True)
            yield
            nc.scalar.copy(pad_view(v_pad, c)[:],
                           vps[:].rearrange("p (r w) -> p r w",
                                            r=ROWS_PER_CHUNK, w=W))  # ACT
            yield

        def scores_gen(c):
            q_v = q_tiles[c][:].rearrange("p (r w) -> p r w", r=ROWS_PER_CHUNK, w=W)
            sc = ps_s.tile([NH * K2, CHUNK], f32, tag="pss")
            mm(sc[:], wqs, z_tiles[c][:], start=True, stop=False)
            yield
            for k in range(K2):
                pk = pkp.tile([CIN, CHUNK], bf16, tag="pk")
                pk_v = pk[:].rearrange("p (r w) -> p r w", r=ROWS_PER_CHUNK, w=W)
                eng = nc.gpsimd if k == 0 else nc.vector
                eng.tensor_tensor(pk_v[:], q_v[:],
                                  pad_view(k_pad, c, _shift_delta(k))[:], mult)
                yield
                mm(sc[:], cb16[:, 420 + k * NH * K2:420 + (k + 1) * NH * K2],
                   pk[:], start=False, stop=(k == K2 - 1))
                yield
            exp_c = smallp.tile([NH * K2, CHUNK], bf16, tag="exp")
            nc.scalar.activation(exp_c[:], sc[:], AF.Exp, bias=bqsc)  # ACT
            yield
            dn = ps_s.tile([NH * K2, CHUNK], f32, tag="pss")
            mm(dn[:], rsw, exp_c[:], start=True, stop=True)
            yield
            rcp = smallp.tile([NH * K2, CHUNK], f32, tag="rcp")
            nc.vector.reciprocal_approx_fast(rcp[:], dn[:])
            yield
            rcp_bf = smallp.tile([NH * K2, CHUNK], bf16, tag="rcpbf")
            nc.scalar.copy(rcp_bf[:], rcp[:])                       # ACT
            yield
            attn_c = smallp.tile([NH * K2, CHUNK], bf16, tag="attn")
            nc.vector.tensor_tensor(attn_c[:], exp_c[:], rcp_bf[:], mult)
            attn_tiles[c] = attn_c
            yield

        def av_gen(c, split=False):
            sl = slice(c * CHUNK, (c + 1) * CHUNK)
            attn_c = attn_tiles[c]
            acc = ps_o.tile([COUT, CHUNK], f32, tag="acc")
            acc2 = None
            if split:
                acc2 = ps_a.tile([COUT, CHUNK], f32, tag="ps_a")
            # per-k: 'a' ACT evict + Pool mult; 'b' ACT evict + DVE mult;
            #        'd' DVE mult straight from PSUM (GPSIMD can't touch PSUM)
            modes = ['a', 'd', 'a', 'd', 'b', 'a', 'd', 'a', 'd']
            for k in range(K2):
                rep = ps_r.tile([CIN, CHUNK], f32, tag="rep")
                mm(rep[:], cb16[0:NH * K2, 873 + k * CIN:873 + (k + 1) * CIN],
                   attn_c[:], start=True, stop=True)
                yield
                mk = mkp.tile([CIN, CHUNK], bf16, tag="mk")
                mk_v = mk[:].rearrange("p (r w) -> p r w", r=ROWS_PER_CHUNK, w=W)
                vv = pad_view(v_pad, c, _shift_delta(k))
                if modes[k] == 'd':
                    nc.vector.tensor_tensor(
                        mk_v[:], rep[:].rearrange("p (r w) -> p r w",
                                                  r=ROWS_PER_CHUNK, w=W),
                        vv[:], mult)
                    yield
                else:
                    rep_sb = repp.tile([CIN, CHUNK], bf16, tag="repsb")
                    nc.scalar.copy(rep_sb[:], rep[:])
                    yield
                    meng = nc.gpsimd if modes[k] == 'a' else nc.vector
                    meng.tensor_tensor(
                        mk_v[:], rep_sb[:].rearrange("p (r w) -> p r w",
                                                     r=ROWS_PER_CHUNK, w=W),
                        vv[:], mult)
                    yield
                if split and k % 2 == 1:
                    mm(acc2[:], wf, mk[:], start=(k == 1), stop=(k == K2 - 2))
                else:
                    mm(acc[:], wf, mk[:], start=(k == 0), stop=(k == K2 - 1))
                yield
            out_sb = outp.tile([COUT, CHUNK], f32, tag="outsb")
            nc.scalar.add(out_sb[:], acc[:], bfb)                # ACT
            if split:
                yield
                nc.vector.tensor_tensor(out_sb[:], out_sb[:], acc2[:],
                                        mybir.AluOpType.add)
            yield
            nc.sync.dma_start(out_ext[:, sl], out_sb[:])
            yield

        def run_all(gens):
            gens = [g for g in gens if g is not None]
            while gens:
                alive = []
                for g in gens:
                    try:
                        next(g)
                        alive.append(g)
                    except StopIteration:
                        pass
                gens = alive

        attn_tiles = [None] * NCHUNK
        for qr in range(4):
            run_all([stats_quarter(qr)])
        run_all([proj_gen(0)])
        run_all([proj_gen(1)])
        run_all([scores_gen(0), proj_gen(2)])
        # steady 2-deep software pipeline: SCORES(c) | AV(c-1) | PROJ(c+2)
        for c in range(1, NCHUNK):
            run_all([scores_gen(c), av_gen(c - 1),
                     proj_gen(c + 2) if c + 2 < NCHUNK else None])
        run_all([av_gen(NCHUNK - 1, split=True)])


def _get_compiled():
    if "nc" not in _CACHE:
        _CACHE["nc"] = _build_bass()
    return _CACHE["nc"]


def kernel(**inputs):
    x = np.asarray(inputs["x"], dtype=np.float32)          # [B, CIN, H, W]
    consts = _host_fold(
        np.asarray(inputs["ln_g"]), np.asarray(inputs["ln_b"]),
        np.asarray(inputs["Wq"]), np.asarray(inputs["bq"]),
        np.asarray(inputs["Wk"]), np.asarray(inputs["bk"]),
        np.asarray(inputs["Wv"]), np.asarray(inputs["bv"]),
        np.asarray(inputs["Wp"]), np.asarray(inputs["bp"]),
        np.asarray(inputs["Wf"]), np.asarray(inputs["bf"]),
    )

    nc = _get_compiled()

    from concourse.bass_utils import run_bass_kernel_spmd

    core_ids = list(range(B))
    in_maps = []
    for i in range(B):
        m = {"x": np.ascontiguousarray(x[i].reshape(CIN, NPIX))}
        m.update(consts)
        in_maps.append(m)

    res = run_bass_kernel_spmd(nc, in_maps, core_ids,
                               trace=bool(int(os.environ.get("KTRACE", "0"))))
    _CACHE["last_result"] = res
    out = np.stack([res.results[i]["out"].reshape(COUT, H, W) for i in range(B)])
    return out.astype(np.float32)


if __name__ == "__main__":
    nc = _get_compiled()
    print("compiled OK")


# revision 7
# speedup vs baseline: 1.2484x; 1.0507x over previous
"""AttentionConv2D (3x3 windowed multi-head attention) on 8 TRN2 NeuronCores. v2.

Sharding: data-parallel over batch (B=8 -> 1 image per core), weights replicated.
Per-core layout: channel-major [128 ch, 4096 pix].

v2 vs v1: no aug matmuls (pre-centered z via DMA partition-broadcast of mu/rstd
rows), biases folded into ACT evictions / Exp bias, SBUF-SBUF stats
redistribution, engine-balanced elementwise work, software-pipelined chunks.
"""

import math
import os
import sys

import numpy as np

sys.path.insert(0, "/opt/trn_rl_repo")

import ml_dtypes  # noqa: E402

BF16 = ml_dtypes.bfloat16

B, CIN, COUT, H, W, KS, NH = 8, 128, 128, 64, 64, 3, 4
A = CIN // NH          # 32
OSH = COUT // NH       # 32
K2 = KS * KS           # 9
NPIX = H * W           # 4096
PW = W + 2             # 66 padded width
PH = H + 2
NPAD = PW * PH + PW + 2  # slack so shifted strided views stay in-bounds
NCHUNK = 8
CHUNK = NPIX // NCHUNK  # 512
ROWS_PER_CHUNK = H // NCHUNK  # 8
SCALE = A ** (-0.5)
PCK = NPIX // CIN      # 32 packed-stat columns per row

_CACHE = {}


def _pos_encoding_np():
    pos = np.arange(K2, dtype=np.float32)[:, None]
    div = np.exp(np.arange(0, CIN, 2, dtype=np.float32) * (-math.log(10000.0) / CIN))
    ang = pos * div[None, :]
    return np.stack([np.sin(ang), np.cos(ang)], -1).reshape(K2, CIN)


def _host_fold(ln_g, ln_b, Wq, bq, Wk, bk, Wv, bv, Wp, bp, Wf, bf):
    """All weight-space precomputation (f64 for accuracy, cast at the end)."""
    g = ln_g.astype(np.float64)
    b = ln_b.astype(np.float64)
    Wq = Wq.astype(np.float64); Wk = Wk.astype(np.float64)
    Wv = Wv.astype(np.float64); Wp = Wp.astype(np.float64)
    Wf = Wf.astype(np.float64)
    bq = bq.astype(np.float64); bk = bk.astype(np.float64)
    bv = bv.astype(np.float64); bp = bp.astype(np.float64)
    bfv = bf.astype(np.float64)

    Wq_ = g[:, None] * Wq; bq_ = b @ Wq + bq
    Wk_ = g[:, None] * Wk; bk_ = b @ Wk + bk
    Wv_ = g[:, None] * Wv; bv_ = b @ Wv + bv

    pos = _pos_encoding_np().astype(np.float64) @ Wp + bp  # [K2, NH*A]
    pos = pos.reshape(K2, NH, A)

    # pos-scores: row layout (n,k) = n*9+k ; scores_pos = z @ Wqs + bqs, scaled
    Wqs = np.zeros((CIN, NH * K2))
    bqs = np.zeros((NH * K2,))
    Wq_r = Wq_.reshape(CIN, NH, A)
    bq_r = bq_.reshape(NH, A)
    for n in range(NH):
        for k in range(K2):
            Wqs[:, n * K2 + k] = Wq_r[:, n, :] @ pos[k, n, :]
            bqs[n * K2 + k] = bq_r[n, :] @ pos[k, n, :]
    Wqs *= SCALE
    bqs *= SCALE

    # BD_k [CIN, 36]: (n,a) x (n*9+k) = SCALE ; concat over k -> [128, 9*36]
    bd = np.zeros((K2, CIN, NH * K2))
    for k in range(K2):
        for n in range(NH):
            bd[k, n * A:(n + 1) * A, n * K2 + k] = SCALE
    bd = np.concatenate([bd[k] for k in range(K2)], axis=1)  # [128, 324]

    # E_k [36, 128]: (n*9+k') x (n,o) = 1 iff k'==k ; concat -> [36, 9*128]
    ek = np.zeros((K2, NH * K2, CIN))
    for k in range(K2):
        for n in range(NH):
            ek[k, n * K2 + k, n * OSH:(n + 1) * OSH] = 1.0
    ek = np.concatenate([ek[k] for k in range(K2)], axis=1)  # [36, 1152]

    # RS36 [36, 36]: (n*9+k) x (n'*9+k') = 1 iff n==n'  (sum over k, rep over k')
    rs = np.zeros((NH * K2, NH * K2))
    for n in range(NH):
        rs[n * K2:(n + 1) * K2, n * K2:(n + 1) * K2] = 1.0

    def pad128(m):
        out = np.zeros((CIN, m.shape[1]))
        out[:m.shape[0]] = m
        return out

    # one concatenated bf16 const blob [128, 2061]:
    # wq(0:128) wk(128:256) wv(256:384) wqs(384:420) bd(420:744) wf(744:872)
    # ones(872:873) ek(873:2025) rs(2025:2061)
    cb16 = np.concatenate([
        Wq_, Wk_, Wv_, Wqs, bd, Wf, np.ones((CIN, 1)), pad128(ek), pad128(rs),
        np.eye(CIN), np.ones((CIN, CIN)),
    ], axis=1).astype(BF16)
    # f32 bias blob [128, 5]: bqc bkc bvc bfb bqsc(pad)
    bfv2 = bfv + (1.0 + K2 * 1e-8) * (bv_ @ Wf)
    cf32 = np.stack([
        bq_, bk_, bv_, bfv2, np.concatenate([bqs, np.zeros(CIN - NH * K2)]),
        np.ones(CIN),
    ], axis=1).astype(np.float32)
    return {"cb16": np.ascontiguousarray(cb16), "cf32": np.ascontiguousarray(cf32)}


def _shift_delta(k):
    di, dj = k // KS - 1, k % KS - 1
    return di * PW + dj


def _build_bass():
    import concourse.bass as bass
    import concourse.tile as tile
    from concourse import bacc, mybir

    f32 = mybir.dt.float32
    bf16 = mybir.dt.bfloat16
    AF = mybir.ActivationFunctionType

    nc = bacc.Bacc("TRN2", target_bir_lowering=False, debug=False)

    ext = {}
    ext["x"] = nc.dram_tensor("x", [CIN, NPIX], f32, kind="ExternalInput")
    ext["cb16"] = nc.dram_tensor("cb16", [CIN, 2317], bf16, kind="ExternalInput")
    ext["cf32"] = nc.dram_tensor("cf32", [CIN, 6], f32, kind="ExternalInput")
    out_ext = nc.dram_tensor("out", [COUT, NPIX], f32, kind="ExternalOutput")

    with tile.TileContext(nc) as tc:
        _kernel_body(tc, nc, mybir, f32, bf16, AF, bass, ext, out_ext)

    nc.compile()
    return nc


def _kernel_body(tc, nc, mybir, f32, bf16, AF, bass, ext, out_ext):
    from contextlib import ExitStack

    f32r = mybir.dt.float32r
    mult = mybir.AluOpType.mult
    sub = mybir.AluOpType.subtract

    ctx = ExitStack()
    with ctx:
        consts = ctx.enter_context(tc.tile_pool(name="consts", bufs=1))
        big = ctx.enter_context(tc.tile_pool(name="big", bufs=1))
        xbfp = ctx.enter_context(tc.tile_pool(name="xbf", bufs=2))
        sqp = ctx.enter_context(tc.tile_pool(name="sqp", bufs=2))
        tmpp = ctx.enter_context(tc.tile_pool(name="tmpp", bufs=3))
        zp = ctx.enter_context(tc.tile_pool(name="zp", bufs=4))
        qp_pool = ctx.enter_context(tc.tile_pool(name="qpool", bufs=4))
        pkp = ctx.enter_context(tc.tile_pool(name="pkp", bufs=6))
        mkp = ctx.enter_context(tc.tile_pool(name="mkp", bufs=6))
        repp = ctx.enter_context(tc.tile_pool(name="repp", bufs=6))
        smallp = ctx.enter_context(tc.tile_pool(name="small", bufs=3))
        statp = ctx.enter_context(tc.tile_pool(name="statp", bufs=1))
        dramp = ctx.enter_context(tc.tile_pool(name="drams", bufs=1, space="DRAM"))
        outp = ctx.enter_context(tc.tile_pool(name="outp", bufs=2))
        ps_a = ctx.enter_context(tc.tile_pool(name="ps_a", bufs=1, space="PSUM"))
        ps_s = ctx.enter_context(tc.tile_pool(name="ps_s", bufs=2, space="PSUM"))
        ps_r = ctx.enter_context(tc.tile_pool(name="ps_r", bufs=3, space="PSUM"))
        ps_o = ctx.enter_context(tc.tile_pool(name="ps_o", bufs=2, space="PSUM"))

        def mm(out, lhsT, rhs, **kw):
            nc.tensor.matmul(out, lhsT, rhs, **kw)

        # ---- big SBUF buffers ----
        x_sb = big.tile([CIN, NPIX], f32)
        k_pad = big.tile([CIN, NPAD], bf16)
        v_pad = big.tile([CIN, NPAD], bf16)
        smb = big.tile([CIN, 2 * NPIX], bf16)  # [rstd | mu] broadcast cols

        # ---- preload ACT tables with dummy ops on a zeroed scratch ----
        scr = statp.tile([1, 4], f32, tag="scr")
        nc.vector.memset(scr[:], 1.0)
        nc.scalar.square(scr[:, 1:2], scr[:, 0:1])
        nc.scalar.sqrt(scr[:, 2:3], scr[:, 0:1])
        nc.scalar.activation(scr[:, 3:4], scr[:, 0:1], AF.Exp)
        nc.scalar.copy(scr[:, 1:2], scr[:, 0:1])
        nc.scalar.add(scr[:, 2:3], scr[:, 0:1], scr[:, 0:1])

        # ---- input + constants (x quarter 0 first, then consts) ----
        nc.scalar.dma_start(out=x_sb[:, 0:CHUNK], in_=ext["x"][:, 0:CHUNK])
        nc.scalar.dma_start(out=x_sb[:, CHUNK:NPIX // 4],
                            in_=ext["x"][:, CHUNK:NPIX // 4])
        cb16 = consts.tile([CIN, 2317], bf16)
        nc.sync.dma_start(cb16[:], ext["cb16"][:])
        cf32 = consts.tile([CIN, 6], f32)
        nc.sync.dma_start(cf32[:], ext["cf32"][:])
        for qx in range(1, 4):
            sl = slice(qx * NPIX // 4, (qx + 1) * NPIX // 4)
            nc.scalar.dma_start(out=x_sb[:, sl], in_=ext["x"][:, sl])
        wq = cb16[:, 0:128]
        wk = cb16[:, 128:256]
        wv = cb16[:, 256:384]
        wqs = cb16[:, 384:420]
        bdw = cb16[:, 420:744]
        wf = cb16[:, 744:872]
        ones_k = cb16[:, 872:873]
        ekw = cb16[0:NH * K2, 873:2025]
        rsw = cb16[0:NH * K2, 2025:2061]
        ident = cb16[:, 2061:2189]
        ones_row = cb16[0:1, 2189:2317]
        ident16 = cb16[0:1, 2061:2062]
        bqc = cf32[:, 0:1]
        bkc = cf32[:, 1:2]
        bvc = cf32[:, 2:3]
        bfb = cf32[:, 3:4]
        bqsc = cf32[0:NH * K2, 4:5]
        one32 = cf32[0:1, 5:6]
        ones32r = cf32[:, 5:6].bitcast(mybir.dt.float32r)

        # ---- stats, issued per quarter so LN finalize overlaps later chunks ----
        # s12row: single row, s1 at [0, j], s2 at [0, NPIX + j]
        s12row = statp.tile([1, 2 * NPIX], f32, tag="s12row")
        s_dram = dramp.tile([2, NPIX], bf16)
        QPIX = NPIX // 4      # 1024 pixels per quarter
        QCK = QPIX // CIN     # 8 packed columns per quarter

        def stats_chunk(c):
            sl = slice(c * CHUNK, (c + 1) * CHUNK)
            x_bf = xbfp.tile([CIN, CHUNK], bf16, tag="xbf")
            nc.gpsimd.tensor_copy(x_bf[:], x_sb[:, sl])            # Pool
            yield
            sq_bf = sqp.tile([CIN, CHUNK], bf16, tag="sq")
            nc.scalar.square(sq_bf[:], x_bf[:])                    # ACT
            yield
            s1 = ps_s.tile([1, CHUNK], f32, tag="pss")
            mm(s1[:], ones_k, x_bf[:], start=True, stop=True)
            yield
            s2 = ps_s.tile([1, CHUNK], f32, tag="pss")
            mm(s2[:], ones_k, sq_bf[:], start=True, stop=True)
            yield
            nc.vector.tensor_copy(s12row[0:1, sl], s1[:])          # DVE evict
            yield
            s2dst = s12row[0:1, NPIX + c * CHUNK:NPIX + (c + 1) * CHUNK]
            if c % 2 == 0:
                nc.scalar.copy(s2dst, s2[:])                       # ACT evict
            else:
                nc.vector.tensor_copy(s2dst, s2[:])                # DVE evict
            yield

        def stats_quarter(qr):
            yield from stats_chunk(2 * qr)
            yield from stats_chunk(2 * qr + 1)
            yield from stats_finalize(qr)

        def fin_pe(g):
            # PE-path LN finalize for head chunk g (pixels g*512..g*512+511):
            # pack via transposes, math, transpose rows back, bcast matmuls
            # into PSUM (rbps/mbps) read directly by the centering ops.
            GC = 4  # 512 px / 128
            base = g * CHUNK
            tps = ps_o.tile([CIN, 2 * GC], f32, tag="acc")
            for j in range(GC):
                o1 = base + j * CIN
                nc.tensor.transpose(tps[:, j:j + 1],
                                    s12row[0:1, o1:o1 + CIN], one32)
                o2 = NPIX + base + j * CIN
                nc.tensor.transpose(tps[:, GC + j:GC + j + 1],
                                    s12row[0:1, o2:o2 + CIN], one32)
            yield
            S1 = tps[:, 0:GC]
            S2 = tps[:, GC:2 * GC]
            stat2 = statp.tile([CIN, 3 * GC], f32, tag=f"fpe{g}")
            mean = stat2[:, 0:GC]
            msq = stat2[:, GC:2 * GC]
            var = stat2[:, 2 * GC:3 * GC]
            nc.vector.tensor_scalar_mul(mean[:], S1[:], 1.0 / CIN)
            yield
            nc.vector.tensor_tensor(msq[:], mean[:], mean[:], mult)
            nc.vector.scalar_tensor_tensor(var[:], S2[:], 1.0 / CIN, msq[:],
                                           mult, sub)
            nc.vector.tensor_scalar_add(var[:], var[:], 1e-5)
            yield
            stdg = statp.tile([CIN, GC], f32, tag=f"fpestd{g}")
            nc.scalar.sqrt(stdg[:], var[:])
            yield
            rstdg = statp.tile([CIN, GC], f32, tag=f"fper{g}")
            nc.vector.reciprocal_approx_fast(rstdg[:], stdg[:])
            yield
            sbfg = statp.tile([CIN, 2 * GC], bf16, tag=f"fpeb{g}")
            nc.vector.tensor_copy(sbfg[:, 0:GC], rstdg[:])
            nc.vector.tensor_copy(sbfg[:, GC:2 * GC], mean[:])
            yield
            # rows: T[j,p]: j 0-3 rstd segments, 4-7 mean segments
            tr = ps_o.tile([2 * GC, CIN], bf16, tag="acc")
            nc.tensor.transpose(tr[:], sbfg[:], ident)
            yield
            srow8 = statp.tile([1, 2 * GC * CIN], bf16, tag=f"fpes{g}")
            engs = [nc.scalar, nc.vector, nc.gpsimd]
            for j in range(2 * GC):
                eng = engs[j % 3]
                if eng is nc.vector:
                    eng.tensor_copy(srow8[0:1, j * CIN:(j + 1) * CIN],
                                    tr[j:j + 1, :])
                elif eng is nc.gpsimd:
                    eng.tensor_copy(srow8[0:1, j * CIN:(j + 1) * CIN],
                                    tr[j:j + 1, :])
                else:
                    eng.copy(srow8[0:1, j * CIN:(j + 1) * CIN], tr[j:j + 1, :])
            yield
            rbp = ps_r.tile([CIN, CHUNK], f32, tag="rep")
            mbp = ps_r.tile([CIN, CHUNK], f32, tag="rep")
            rbps[g], mbps[g] = rbp, mbp
            for j in range(GC):
                mm(rbp[:, j * CIN:(j + 1) * CIN], ones_row,
                   srow8[0:1, j * CIN:(j + 1) * CIN], start=True, stop=True)
                mm(mbp[:, j * CIN:(j + 1) * CIN], ones_row,
                   srow8[0:1, (GC + j) * CIN:(GC + j + 1) * CIN],
                   start=True, stop=True)
            yield

        def transpose_pack(qr, tps, half):
            for j in range(half * QCK // 2, (half + 1) * QCK // 2):
                o1 = qr * QPIX + j * CIN
                nc.tensor.transpose(tps[:, j:j + 1],
                                    s12row[0:1, o1:o1 + CIN], one32)
                o2 = NPIX + qr * QPIX + j * CIN
                nc.tensor.transpose(tps[:, QCK + j:QCK + j + 1],
                                    s12row[0:1, o2:o2 + CIN], one32)

        def stats_finalize(qr):
            qsl = slice(qr * QPIX, (qr + 1) * QPIX)
            qsl2 = slice(NPIX + qr * QPIX, NPIX + (qr + 1) * QPIX)
            # pack quarter via PE transposes: tps[p, b*QCK+j] = s_b[qr*1024+j*128+p]
            tps = ps_s.tile([CIN, 2 * QCK], f32, tag="pss")
            transpose_pack(qr, tps, 0)
            yield
            transpose_pack(qr, tps, 1)
            yield
            S1 = tps[:, 0:QCK]
            S2 = tps[:, QCK:2 * QCK]
            stat2 = statp.tile([CIN, 3 * QCK], f32, tag=f"stat2{qr}")
            mean = stat2[:, 0:QCK]
            msq = stat2[:, QCK:2 * QCK]
            var = stat2[:, 2 * QCK:3 * QCK]
            nc.vector.tensor_scalar_mul(mean[:], S1[:], 1.0 / CIN)
            yield
            nc.vector.tensor_tensor(msq[:], mean[:], mean[:], mult)
            nc.vector.scalar_tensor_tensor(var[:], S2[:], 1.0 / CIN, msq[:], mult, sub)
            nc.vector.tensor_scalar_add(var[:], var[:], 1e-5)
            yield
            std = statp.tile([CIN, QCK], f32, tag=f"std{qr}")
            nc.scalar.sqrt(std[:], var[:])
            rstd32 = statp.tile([CIN, QCK], f32, tag=f"rstd32{qr}")
            nc.vector.reciprocal_approx_fast(rstd32[:], std[:])
            stat_bf = statp.tile([CIN, 2 * QCK], bf16, tag=f"stat_bf{qr}")
            nc.vector.tensor_copy(stat_bf[:, 0:QCK], rstd32[:])
            yield
            nc.vector.tensor_copy(stat_bf[:, QCK:2 * QCK], mean[:])
            yield
            # DMAs to DRAM rows; pixel index = qr*1024 + j*128 + p
            dd0 = s_dram[0:1, 0:1]
            for row, scols in ((0, slice(0, QCK)), (1, slice(QCK, 2 * QCK))):
                ddst = bass.AP(tensor=dd0.tensor,
                               offset=dd0.offset + row * NPIX + qr * QPIX,
                               ap=[[1, CIN], [CIN, QCK]])
                nc.sync.dma_start(ddst, stat_bf[:, scols])
                yield
            # partition-broadcast back into smb ([rstd | mu] column blocks)
            dd = s_dram[0:1, 0:1]
            for row, dcols in ((0, qsl), (1, qsl2)):
                src = bass.AP(tensor=dd.tensor,
                              offset=dd.offset + row * NPIX + qr * QPIX,
                              ap=[[0, CIN], [1, QPIX]])
                nc.sync.dma_start(smb[:, dcols], src)
                yield

        for pad_t in (k_pad, v_pad):
            nc.gpsimd.memset(pad_t[:, 0:PW + 1], 0.0)
            nc.gpsimd.memset(
                pad_t[:, PW + 65:PW + 65 + 64 * PW].rearrange(
                    "p (r t) -> p r t", t=PW)[:, :, 0:2], 0.0)
            nc.gpsimd.memset(pad_t[:, 65 * PW + 1:NPAD], 0.0)

        z_tiles = [None] * NCHUNK
        q_tiles = [None] * NCHUNK

        def pad_view(t, c, delta=0):
            off = (1 + c * ROWS_PER_CHUNK) * PW + 1 + delta
            return t[:, off:off + ROWS_PER_CHUNK * PW].rearrange(
                "p (r w) -> p r w", r=ROWS_PER_CHUNK, w=PW)[:, :, 0:W]

        def proj_gen(c):
            sl = slice(c * CHUNK, (c + 1) * CHUNK)
            tmp = tmpp.tile([CIN, CHUNK], bf16, tag="tmp")
            nc.vector.tensor_tensor(
                tmp[:], x_sb[:, sl],
                smb[:, NPIX + c * CHUNK:NPIX + (c + 1) * CHUNK], sub)
            yield
            z = zp.tile([CIN, CHUNK], bf16, tag="z")
            z_tiles[c] = z
            nc.vector.tensor_tensor(z[:], tmp[:], smb[:, sl], mult)
            yield
            qps = ps_a.tile([CIN, CHUNK], f32, tag="ps_a")
            mm(qps[:], wq, z[:], start=True, stop=True)
            yield
            q_c = qp_pool.tile([CIN, CHUNK], bf16, tag="q")
            q_tiles[c] = q_c
            nc.scalar.add(q_c[:], qps[:], bqc)                  # ACT
            yield
            kps = ps_a.tile([CIN, CHUNK], f32, tag="ps_a")
            mm(kps[:], wk, z[:], start=True, stop=True)
            yield
            nc.scalar.add(pad_view(k_pad, c)[:],
                          kps[:].rearrange("p (r w) -> p r w",
                                           r=ROWS_PER_CHUNK, w=W), bkc)  # ACT
            yield
            vps = ps_a.tile([CIN, CHUNK], f32, tag="ps_a")
            mm(vps[:], wv, z[:], start=True, stop=True)
            yield
            nc.scalar.copy(pad_view(v_pad, c)[:],
                           vps[:].rearrange("p (r w) -> p r w",
                                            r=ROWS_PER_CHUNK, w=W))  # ACT
            yield

        def scores_gen(c):
            q_v = q_tiles[c][:].rearrange("p (r w) -> p r w", r=ROWS_PER_CHUNK, w=W)
            sc = ps_s.tile([NH * K2, CHUNK], f32, tag="pss")
            mm(sc[:], wqs, z_tiles[c][:], start=True, stop=False)
            yield
            for k in range(K2):
                pk = pkp.tile([CIN, CHUNK], bf16, tag="pk")
                pk_v = pk[:].rearrange("p (r w) -> p r w", r=ROWS_PER_CHUNK, w=W)
                eng = nc.gpsimd if k >= 5 else nc.vector
                eng.tensor_tensor(pk_v[:], q_v[:],
                                  pad_view(k_pad, c, _shift_delta(k))[:], mult)
                yield
                mm(sc[:], cb16[:, 420 + k * NH * K2:420 + (k + 1) * NH * K2],
                   pk[:], start=False, stop=(k == K2 - 1))
                yield
            exp_c = smallp.tile([NH * K2, CHUNK], bf16, tag="exp")
            nc.scalar.activation(exp_c[:], sc[:], AF.Exp, bias=bqsc)  # ACT
            yield
            dn = ps_s.tile([NH * K2, CHUNK], f32, tag="pss")
            mm(dn[:], rsw, exp_c[:], start=True, stop=True)
            yield
            rcp = smallp.tile([NH * K2, CHUNK], f32, tag="rcp")
            nc.vector.reciprocal_approx_fast(rcp[:], dn[:])
            yield
            attn_c = smallp.tile([NH * K2, CHUNK], bf16, tag="attn")
            nc.gpsimd.tensor_tensor(attn_c[:], exp_c[:], rcp[:], mult)
            attn_tiles[c] = attn_c
            yield

        def av_gen(c, split=False):
            sl = slice(c * CHUNK, (c + 1) * CHUNK)
            attn_c = attn_tiles[c]
            acc = ps_o.tile([COUT, CHUNK], f32, tag="acc")
            acc2 = None
            if split:
                acc2 = ps_a.tile([COUT, CHUNK], f32, tag="ps_a")
            # rep matmuls write bf16 PSUM (exact: 0/1 matrix x bf16 attn);
            # mk multiplies read PSUM directly as 2-byte packed operands.
            # 'e': ACT-evict + DVE bf16 mult for a couple of ks to offload DVE.
            modes = ['e', 'd', 'e', 'd', 'e', 'e', 'd', 'e', 'e']
            for k in range(K2):
                rep = ps_r.tile([CIN, CHUNK], f32, tag="rep")
                mm(rep[:], cb16[0:NH * K2, 873 + k * CIN:873 + (k + 1) * CIN],
                   attn_c[:], start=True, stop=True)
                yield
                mk = mkp.tile([CIN, CHUNK], bf16, tag="mk")
                mk_v = mk[:].rearrange("p (r w) -> p r w", r=ROWS_PER_CHUNK, w=W)
                vv = pad_view(v_pad, c, _shift_delta(k))
                if modes[k] == 'd':
                    nc.vector.tensor_tensor(
                        mk_v[:], rep[:].rearrange("p (r w) -> p r w",
                                                  r=ROWS_PER_CHUNK, w=W),
                        vv[:], mult)
                    yield
                else:
                    rep_sb = repp.tile([CIN, CHUNK], bf16, tag="repsb")
                    nc.scalar.copy(rep_sb[:], rep[:])
                    yield
                    nc.vector.tensor_tensor(
                        mk_v[:], rep_sb[:].rearrange("p (r w) -> p r w",
                                                     r=ROWS_PER_CHUNK, w=W),
                        vv[:], mult)
                    yield
                if split and k % 2 == 1:
                    mm(acc2[:], wf, mk[:], start=(k == 1), stop=(k == K2 - 2))
                else:
                    mm(acc[:], wf, mk[:], start=(k == 0), stop=(k == K2 - 1))
                yield
            out_sb = outp.tile([COUT, CHUNK], f32, tag="outsb")
            nc.scalar.add(out_sb[:], acc[:], bfb)                # ACT
            if split:
                yield
                nc.vector.tensor_tensor(out_sb[:], out_sb[:], acc2[:],
                                        mybir.AluOpType.add)
            yield
            nc.sync.dma_start(out_ext[:, sl], out_sb[:])
            yield

        def run_all(gens):
            gens = [g for g in gens if g is not None]
            while gens:
                alive = []
                for g in gens:
                    try:
                        next(g)
                        alive.append(g)
                    except StopIteration:
                        pass
                gens = alive

        attn_tiles = [None] * NCHUNK
        for qr in range(4):
            run_all([stats_quarter(qr)])
        run_all([proj_gen(0)])
        run_all([proj_gen(1)])
        run_all([scores_gen(0), proj_gen(2)])
        # steady 2-deep software pipeline: SCORES(c) | AV(c-1) | PROJ(c+2)
        for c in range(1, NCHUNK):
            run_all([scores_gen(c), av_gen(c - 1),
                     proj_gen(c + 2) if c + 2 < NCHUNK else None])
        run_all([av_gen(NCHUNK - 1, split=True)])


def _get_compiled():
    if "nc" not in _CACHE:
        _CACHE["nc"] = _build_bass()
    return _CACHE["nc"]


def kernel(**inputs):
    x = np.asarray(inputs["x"], dtype=np.float32)          # [B, CIN, H, W]
    consts = _host_fold(
        np.asarray(inputs["ln_g"]), np.asarray(inputs["ln_b"]),
        np.asarray(inputs["Wq"]), np.asarray(inputs["bq"]),
        np.asarray(inputs["Wk"]), np.asarray(inputs["bk"]),
        np.asarray(inputs["Wv"]), np.asarray(inputs["bv"]),
        np.asarray(inputs["Wp"]), np.asarray(inputs["bp"]),
        np.asarray(inputs["Wf"]), np.asarray(inputs["bf"]),
    )

    nc = _get_compiled()

    from concourse.bass_utils import run_bass_kernel_spmd

    core_ids = list(range(B))
    in_maps = []
    for i in range(B):
        m = {"x": np.ascontiguousarray(x[i].reshape(CIN, NPIX))}
        m.update(consts)
        in_maps.append(m)

    res = run_bass_kernel_spmd(nc, in_maps, core_ids,
                               trace=bool(int(os.environ.get("KTRACE", "0"))))
    _CACHE["last_result"] = res
    out = np.stack([res.results[i]["out"].reshape(COUT, H, W) for i in range(B)])
    return out.astype(np.float32)


if __name__ == "__main__":
    nc = _get_compiled()
    print("compiled OK")


# revision 8
# speedup vs baseline: 1.2863x; 1.0303x over previous
"""AttentionConv2D (3x3 windowed multi-head attention) on 8 TRN2 NeuronCores. v2.

Sharding: data-parallel over batch (B=8 -> 1 image per core), weights replicated.
Per-core layout: channel-major [128 ch, 4096 pix].

v2 vs v1: no aug matmuls (pre-centered z via DMA partition-broadcast of mu/rstd
rows), biases folded into ACT evictions / Exp bias, SBUF-SBUF stats
redistribution, engine-balanced elementwise work, software-pipelined chunks.
"""

import math
import os
import sys

import numpy as np

sys.path.insert(0, "/opt/trn_rl_repo")

import ml_dtypes  # noqa: E402

BF16 = ml_dtypes.bfloat16

B, CIN, COUT, H, W, KS, NH = 8, 128, 128, 64, 64, 3, 4
A = CIN // NH          # 32
OSH = COUT // NH       # 32
K2 = KS * KS           # 9
NPIX = H * W           # 4096
PW = W + 2             # 66 padded width
PH = H + 2
NPAD = PW * PH + PW + 2  # slack so shifted strided views stay in-bounds
NCHUNK = 8
CHUNK = NPIX // NCHUNK  # 512
ROWS_PER_CHUNK = H // NCHUNK  # 8
SCALE = A ** (-0.5)
PCK = NPIX // CIN      # 32 packed-stat columns per row

_CACHE = {}


def _pos_encoding_np():
    pos = np.arange(K2, dtype=np.float32)[:, None]
    div = np.exp(np.arange(0, CIN, 2, dtype=np.float32) * (-math.log(10000.0) / CIN))
    ang = pos * div[None, :]
    return np.stack([np.sin(ang), np.cos(ang)], -1).reshape(K2, CIN)


def _host_fold(ln_g, ln_b, Wq, bq, Wk, bk, Wv, bv, Wp, bp, Wf, bf):
    """All weight-space precomputation (f64 for accuracy, cast at the end)."""
    g = ln_g.astype(np.float64)
    b = ln_b.astype(np.float64)
    Wq = Wq.astype(np.float64); Wk = Wk.astype(np.float64)
    Wv = Wv.astype(np.float64); Wp = Wp.astype(np.float64)
    Wf = Wf.astype(np.float64)
    bq = bq.astype(np.float64); bk = bk.astype(np.float64)
    bv = bv.astype(np.float64); bp = bp.astype(np.float64)
    bfv = bf.astype(np.float64)

    Wq_ = g[:, None] * Wq; bq_ = b @ Wq + bq
    Wk_ = g[:, None] * Wk; bk_ = b @ Wk + bk
    Wv_ = g[:, None] * Wv; bv_ = b @ Wv + bv

    pos = _pos_encoding_np().astype(np.float64) @ Wp + bp  # [K2, NH*A]
    pos = pos.reshape(K2, NH, A)

    # pos-scores: row layout (n,k) = n*9+k ; scores_pos = z @ Wqs + bqs, scaled
    Wqs = np.zeros((CIN, NH * K2))
    bqs = np.zeros((NH * K2,))
    Wq_r = Wq_.reshape(CIN, NH, A)
    bq_r = bq_.reshape(NH, A)
    for n in range(NH):
        for k in range(K2):
            Wqs[:, n * K2 + k] = Wq_r[:, n, :] @ pos[k, n, :]
            bqs[n * K2 + k] = bq_r[n, :] @ pos[k, n, :]
    Wqs *= SCALE
    bqs *= SCALE

    # BD_k [CIN, 36]: (n,a) x (n*9+k) = SCALE ; concat over k -> [128, 9*36]
    bd = np.zeros((K2, CIN, NH * K2))
    for k in range(K2):
        for n in range(NH):
            bd[k, n * A:(n + 1) * A, n * K2 + k] = SCALE
    bd = np.concatenate([bd[k] for k in range(K2)], axis=1)  # [128, 324]

    # E_k [36, 128]: (n*9+k') x (n,o) = 1 iff k'==k ; concat -> [36, 9*128]
    ek = np.zeros((K2, NH * K2, CIN))
    for k in range(K2):
        for n in range(NH):
            ek[k, n * K2 + k, n * OSH:(n + 1) * OSH] = 1.0
    ek = np.concatenate([ek[k] for k in range(K2)], axis=1)  # [36, 1152]

    # RS36 [36, 36]: (n*9+k) x (n'*9+k') = 1 iff n==n'  (sum over k, rep over k')
    rs = np.zeros((NH * K2, NH * K2))
    for n in range(NH):
        rs[n * K2:(n + 1) * K2, n * K2:(n + 1) * K2] = 1.0

    def pad128(m):
        out = np.zeros((CIN, m.shape[1]))
        out[:m.shape[0]] = m
        return out

    # one concatenated bf16 const blob [128, 2061]:
    # wq(0:128) wk(128:256) wv(256:384) wqs(384:420) bd(420:744) wf(744:872)
    # ones(872:873) ek(873:2025) rs(2025:2061)
    cb16 = np.concatenate([
        Wq_, Wk_, Wv_, Wqs, bd, Wf, np.ones((CIN, 1)), pad128(ek), pad128(rs),
        np.eye(CIN), np.ones((CIN, CIN)),
    ], axis=1).astype(BF16)
    # f32 bias blob [128, 5]: bqc bkc bvc bfb bqsc(pad)
    bfv2 = bfv + (1.0 + K2 * 1e-8) * (bv_ @ Wf)
    cf32 = np.stack([
        bq_, bk_, bv_, bfv2, np.concatenate([bqs, np.zeros(CIN - NH * K2)]),
        np.ones(CIN),
    ], axis=1).astype(np.float32)
    return {"cb16": np.ascontiguousarray(cb16), "cf32": np.ascontiguousarray(cf32)}


def _shift_delta(k):
    di, dj = k // KS - 1, k % KS - 1
    return di * PW + dj


def _build_bass():
    import concourse.bass as bass
    import concourse.tile as tile
    from concourse import bacc, mybir

    f32 = mybir.dt.float32
    bf16 = mybir.dt.bfloat16
    AF = mybir.ActivationFunctionType

    nc = bacc.Bacc("TRN2", target_bir_lowering=False, debug=False)

    ext = {}
    ext["x"] = nc.dram_tensor("x", [CIN, NPIX], f32, kind="ExternalInput")
    ext["cb16"] = nc.dram_tensor("cb16", [CIN, 2317], bf16, kind="ExternalInput")
    ext["cf32"] = nc.dram_tensor("cf32", [CIN, 6], f32, kind="ExternalInput")
    out_ext = nc.dram_tensor("out", [COUT, NPIX], f32, kind="ExternalOutput")

    with tile.TileContext(nc) as tc:
        _kernel_body(tc, nc, mybir, f32, bf16, AF, bass, ext, out_ext)

    nc.compile()
    return nc


def _kernel_body(tc, nc, mybir, f32, bf16, AF, bass, ext, out_ext):
    from contextlib import ExitStack

    f32r = mybir.dt.float32r
    mult = mybir.AluOpType.mult
    sub = mybir.AluOpType.subtract

    ctx = ExitStack()
    with ctx:
        consts = ctx.enter_context(tc.tile_pool(name="consts", bufs=1))
        big = ctx.enter_context(tc.tile_pool(name="big", bufs=1))
        xbfp = ctx.enter_context(tc.tile_pool(name="xbf", bufs=2))
        sqp = ctx.enter_context(tc.tile_pool(name="sqp", bufs=2))
        tmpp = ctx.enter_context(tc.tile_pool(name="tmpp", bufs=3))
        zp = ctx.enter_context(tc.tile_pool(name="zp", bufs=4))
        qp_pool = ctx.enter_context(tc.tile_pool(name="qpool", bufs=4))
        pkp = ctx.enter_context(tc.tile_pool(name="pkp", bufs=6))
        mkp = ctx.enter_context(tc.tile_pool(name="mkp", bufs=6))
        repp = ctx.enter_context(tc.tile_pool(name="repp", bufs=6))
        smallp = ctx.enter_context(tc.tile_pool(name="small", bufs=3))
        statp = ctx.enter_context(tc.tile_pool(name="statp", bufs=1))
        dramp = ctx.enter_context(tc.tile_pool(name="drams", bufs=1, space="DRAM"))
        outp = ctx.enter_context(tc.tile_pool(name="outp", bufs=2))
        ps_a = ctx.enter_context(tc.tile_pool(name="ps_a", bufs=1, space="PSUM"))
        ps_s = ctx.enter_context(tc.tile_pool(name="ps_s", bufs=2, space="PSUM"))
        ps_r = ctx.enter_context(tc.tile_pool(name="ps_r", bufs=3, space="PSUM"))
        ps_o = ctx.enter_context(tc.tile_pool(name="ps_o", bufs=2, space="PSUM"))

        def mm(out, lhsT, rhs, **kw):
            nc.tensor.matmul(out, lhsT, rhs, **kw)

        # ---- big SBUF buffers ----
        x_sb = big.tile([CIN, NPIX], f32)
        k_pad = big.tile([CIN, NPAD], bf16)
        v_pad = big.tile([CIN, NPAD], bf16)
        smb = big.tile([CIN, 2 * NPIX], bf16)  # [rstd | mu] broadcast cols

        # ---- preload ACT tables with dummy ops on a zeroed scratch ----
        scr = statp.tile([1, 4], f32, tag="scr")
        nc.vector.memset(scr[:], 1.0)
        nc.scalar.square(scr[:, 1:2], scr[:, 0:1])
        nc.scalar.sqrt(scr[:, 2:3], scr[:, 0:1])
        nc.scalar.activation(scr[:, 3:4], scr[:, 0:1], AF.Exp)
        nc.scalar.copy(scr[:, 1:2], scr[:, 0:1])
        nc.scalar.add(scr[:, 2:3], scr[:, 0:1], scr[:, 0:1])

        # ---- input + constants (x quarter 0 first, then consts) ----
        nc.scalar.dma_start(out=x_sb[:, 0:CHUNK], in_=ext["x"][:, 0:CHUNK])
        nc.scalar.dma_start(out=x_sb[:, CHUNK:NPIX // 4],
                            in_=ext["x"][:, CHUNK:NPIX // 4])
        cb16 = consts.tile([CIN, 2317], bf16)
        nc.sync.dma_start(cb16[:], ext["cb16"][:])
        cf32 = consts.tile([CIN, 6], f32)
        nc.sync.dma_start(cf32[:], ext["cf32"][:])
        for qx in range(1, 4):
            sl = slice(qx * NPIX // 4, (qx + 1) * NPIX // 4)
            nc.scalar.dma_start(out=x_sb[:, sl], in_=ext["x"][:, sl])
        wq = cb16[:, 0:128]
        wk = cb16[:, 128:256]
        wv = cb16[:, 256:384]
        wqs = cb16[:, 384:420]
        bdw = cb16[:, 420:744]
        wf = cb16[:, 744:872]
        ones_k = cb16[:, 872:873]
        ekw = cb16[0:NH * K2, 873:2025]
        rsw = cb16[0:NH * K2, 2025:2061]
        ident = cb16[:, 2061:2189]
        ones_row = cb16[0:1, 2189:2317]
        ident16 = cb16[0:1, 2061:2062]
        bqc = cf32[:, 0:1]
        bkc = cf32[:, 1:2]
        bvc = cf32[:, 2:3]
        bfb = cf32[:, 3:4]
        bqsc = cf32[0:NH * K2, 4:5]
        one32 = cf32[0:1, 5:6]
        ones32r = cf32[:, 5:6].bitcast(mybir.dt.float32r)

        # ---- stats, issued per quarter so LN finalize overlaps later chunks ----
        # s12row: single row, s1 at [0, j], s2 at [0, NPIX + j]
        s12row = statp.tile([1, 2 * NPIX], f32, tag="s12row")
        s_dram = dramp.tile([2, NPIX], bf16)
        QPIX = NPIX // 4      # 1024 pixels per quarter
        QCK = QPIX // CIN     # 8 packed columns per quarter

        def stats_chunk(c):
            sl = slice(c * CHUNK, (c + 1) * CHUNK)
            x_bf = xbfp.tile([CIN, CHUNK], bf16, tag="xbf")
            nc.gpsimd.tensor_copy(x_bf[:], x_sb[:, sl])            # Pool
            yield
            sq_bf = sqp.tile([CIN, CHUNK], bf16, tag="sq")
            nc.scalar.square(sq_bf[:], x_bf[:])                    # ACT
            yield
            s1 = ps_s.tile([1, CHUNK], f32, tag="pss")
            mm(s1[:], ones_k, x_bf[:], start=True, stop=True)
            yield
            s2 = ps_s.tile([1, CHUNK], f32, tag="pss")
            mm(s2[:], ones_k, sq_bf[:], start=True, stop=True)
            yield
            nc.vector.tensor_copy(s12row[0:1, sl], s1[:])          # DVE evict
            yield
            s2dst = s12row[0:1, NPIX + c * CHUNK:NPIX + (c + 1) * CHUNK]
            if c % 2 == 0:
                nc.scalar.copy(s2dst, s2[:])                       # ACT evict
            else:
                nc.vector.tensor_copy(s2dst, s2[:])                # DVE evict
            yield

        def stats_quarter(qr):
            yield from stats_chunk(2 * qr)
            yield from stats_chunk(2 * qr + 1)
            yield from stats_finalize(qr)

        def fin_pe(g):
            # PE-path LN finalize for head chunk g (pixels g*512..g*512+511):
            # pack via transposes, math, transpose rows back, bcast matmuls
            # into PSUM (rbps/mbps) read directly by the centering ops.
            GC = 4  # 512 px / 128
            base = g * CHUNK
            tps = ps_o.tile([CIN, 2 * GC], f32, tag="acc")
            for j in range(GC):
                o1 = base + j * CIN
                nc.tensor.transpose(tps[:, j:j + 1],
                                    s12row[0:1, o1:o1 + CIN], one32)
                o2 = NPIX + base + j * CIN
                nc.tensor.transpose(tps[:, GC + j:GC + j + 1],
                                    s12row[0:1, o2:o2 + CIN], one32)
            yield
            S1 = tps[:, 0:GC]
            S2 = tps[:, GC:2 * GC]
            stat2 = statp.tile([CIN, 3 * GC], f32, tag=f"fpe{g}")
            mean = stat2[:, 0:GC]
            msq = stat2[:, GC:2 * GC]
            var = stat2[:, 2 * GC:3 * GC]
            nc.vector.tensor_scalar_mul(mean[:], S1[:], 1.0 / CIN)
            yield
            nc.vector.tensor_tensor(msq[:], mean[:], mean[:], mult)
            nc.vector.scalar_tensor_tensor(var[:], S2[:], 1.0 / CIN, msq[:],
                                           mult, sub)
            nc.vector.tensor_scalar_add(var[:], var[:], 1e-5)
            yield
            stdg = statp.tile([CIN, GC], f32, tag=f"fpestd{g}")
            nc.scalar.sqrt(stdg[:], var[:])
            yield
            rstdg = statp.tile([CIN, GC], f32, tag=f"fper{g}")
            nc.vector.reciprocal_approx_fast(rstdg[:], stdg[:])
            yield
            sbfg = statp.tile([CIN, 2 * GC], bf16, tag=f"fpeb{g}")
            nc.vector.tensor_copy(sbfg[:, 0:GC], rstdg[:])
            nc.vector.tensor_copy(sbfg[:, GC:2 * GC], mean[:])
            yield
            # rows: T[j,p]: j 0-3 rstd segments, 4-7 mean segments
            tr = ps_o.tile([2 * GC, CIN], bf16, tag="acc")
            nc.tensor.transpose(tr[:], sbfg[:], ident)
            yield
            srow8 = statp.tile([1, 2 * GC * CIN], bf16, tag=f"fpes{g}")
            engs = [nc.scalar, nc.vector, nc.gpsimd]
            for j in range(2 * GC):
                eng = engs[j % 3]
                if eng is nc.vector:
                    eng.tensor_copy(srow8[0:1, j * CIN:(j + 1) * CIN],
                                    tr[j:j + 1, :])
                elif eng is nc.gpsimd:
                    eng.tensor_copy(srow8[0:1, j * CIN:(j + 1) * CIN],
                                    tr[j:j + 1, :])
                else:
                    eng.copy(srow8[0:1, j * CIN:(j + 1) * CIN], tr[j:j + 1, :])
            yield
            rbp = ps_r.tile([CIN, CHUNK], f32, tag="rep")
            mbp = ps_r.tile([CIN, CHUNK], f32, tag="rep")
            rbps[g], mbps[g] = rbp, mbp
            for j in range(GC):
                mm(rbp[:, j * CIN:(j + 1) * CIN], ones_row,
                   srow8[0:1, j * CIN:(j + 1) * CIN], start=True, stop=True)
                mm(mbp[:, j * CIN:(j + 1) * CIN], ones_row,
                   srow8[0:1, (GC + j) * CIN:(GC + j + 1) * CIN],
                   start=True, stop=True)
            yield

        def transpose_pack(qr, tps, half):
            for j in range(half * QCK // 2, (half + 1) * QCK // 2):
                o1 = qr * QPIX + j * CIN
                nc.tensor.transpose(tps[:, j:j + 1],
                                    s12row[0:1, o1:o1 + CIN], one32)
                o2 = NPIX + qr * QPIX + j * CIN
                nc.tensor.transpose(tps[:, QCK + j:QCK + j + 1],
                                    s12row[0:1, o2:o2 + CIN], one32)

        def stats_finalize(qr):
            qsl = slice(qr * QPIX, (qr + 1) * QPIX)
            qsl2 = slice(NPIX + qr * QPIX, NPIX + (qr + 1) * QPIX)
            # pack quarter via PE transposes: tps[p, b*QCK+j] = s_b[qr*1024+j*128+p]
            tps = ps_s.tile([CIN, 2 * QCK], f32, tag="pss")
            transpose_pack(qr, tps, 0)
            yield
            transpose_pack(qr, tps, 1)
            yield
            S1 = tps[:, 0:QCK]
            S2 = tps[:, QCK:2 * QCK]
            stat2 = statp.tile([CIN, 3 * QCK], f32, tag=f"stat2{qr}")
            mean = stat2[:, 0:QCK]
            msq = stat2[:, QCK:2 * QCK]
            var = stat2[:, 2 * QCK:3 * QCK]
            nc.vector.tensor_scalar_mul(mean[:], S1[:], 1.0 / CIN)
            yield
            nc.vector.tensor_tensor(msq[:], mean[:], mean[:], mult)
            nc.vector.scalar_tensor_tensor(var[:], S2[:], 1.0 / CIN, msq[:], mult, sub)
            nc.vector.tensor_scalar_add(var[:], var[:], 1e-5)
            yield
            std = statp.tile([CIN, QCK], f32, tag=f"std{qr}")
            nc.scalar.sqrt(std[:], var[:])
            rstd32 = statp.tile([CIN, QCK], f32, tag=f"rstd32{qr}")
            nc.vector.reciprocal_approx_fast(rstd32[:], std[:])
            stat_bf = statp.tile([CIN, 2 * QCK], bf16, tag=f"stat_bf{qr}")
            nc.vector.tensor_copy(stat_bf[:, 0:QCK], rstd32[:])
            yield
            nc.vector.tensor_copy(stat_bf[:, QCK:2 * QCK], mean[:])
            yield
            # DMAs to DRAM rows; pixel index = qr*1024 + j*128 + p
            dd0 = s_dram[0:1, 0:1]
            for row, scols in ((0, slice(0, QCK)), (1, slice(QCK, 2 * QCK))):
                ddst = bass.AP(tensor=dd0.tensor,
                               offset=dd0.offset + row * NPIX + qr * QPIX,
                               ap=[[1, CIN], [CIN, QCK]])
                nc.sync.dma_start(ddst, stat_bf[:, scols])
                yield
            # partition-broadcast back into smb ([rstd | mu] column blocks)
            dd = s_dram[0:1, 0:1]
            for row, dcols in ((0, qsl), (1, qsl2)):
                src = bass.AP(tensor=dd.tensor,
                              offset=dd.offset + row * NPIX + qr * QPIX,
                              ap=[[0, CIN], [1, QPIX]])
                nc.sync.dma_start(smb[:, dcols], src)
                yield

        for pad_t in (k_pad, v_pad):
            nc.gpsimd.memset(pad_t[:, 0:PW + 1], 0.0)
            nc.gpsimd.memset(
                pad_t[:, PW + 65:PW + 65 + 64 * PW].rearrange(
                    "p (r t) -> p r t", t=PW)[:, :, 0:2], 0.0)
            nc.gpsimd.memset(pad_t[:, 65 * PW + 1:NPAD], 0.0)

        z_tiles = [None] * NCHUNK
        q_tiles = [None] * NCHUNK

        def pad_view(t, c, delta=0):
            off = (1 + c * ROWS_PER_CHUNK) * PW + 1 + delta
            return t[:, off:off + ROWS_PER_CHUNK * PW].rearrange(
                "p (r w) -> p r w", r=ROWS_PER_CHUNK, w=PW)[:, :, 0:W]

        def proj_gen(c):
            sl = slice(c * CHUNK, (c + 1) * CHUNK)
            tmp = tmpp.tile([CIN, CHUNK], bf16, tag="tmp")
            nc.vector.tensor_tensor(
                tmp[:], x_sb[:, sl],
                smb[:, NPIX + c * CHUNK:NPIX + (c + 1) * CHUNK], sub)
            yield
            z = zp.tile([CIN, CHUNK], bf16, tag="z")
            z_tiles[c] = z
            nc.vector.tensor_tensor(z[:], tmp[:], smb[:, sl], mult)
            yield
            qps = ps_a.tile([CIN, CHUNK], f32, tag="ps_a")
            mm(qps[:], wq, z[:], start=True, stop=True)
            yield
            q_c = qp_pool.tile([CIN, CHUNK], bf16, tag="q")
            q_tiles[c] = q_c
            nc.scalar.add(q_c[:], qps[:], bqc)                  # ACT
            yield
            kps = ps_a.tile([CIN, CHUNK], f32, tag="ps_a")
            mm(kps[:], wk, z[:], start=True, stop=True)
            yield
            nc.scalar.add(pad_view(k_pad, c)[:],
                          kps[:].rearrange("p (r w) -> p r w",
                                           r=ROWS_PER_CHUNK, w=W), bkc)  # ACT
            yield
            vps = ps_a.tile([CIN, CHUNK], f32, tag="ps_a")
            mm(vps[:], wv, z[:], start=True, stop=True)
            yield
            nc.scalar.copy(pad_view(v_pad, c)[:],
                           vps[:].rearrange("p (r w) -> p r w",
                                            r=ROWS_PER_CHUNK, w=W))  # ACT
            yield

        def scores_gen(c):
            q_v = q_tiles[c][:].rearrange("p (r w) -> p r w", r=ROWS_PER_CHUNK, w=W)
            sc = ps_s.tile([NH * K2, CHUNK], f32, tag="pss")
            mm(sc[:], wqs, z_tiles[c][:], start=True, stop=False)
            yield
            for k in range(K2):
                pk = pkp.tile([CIN, CHUNK], bf16, tag="pk")
                pk_v = pk[:].rearrange("p (r w) -> p r w", r=ROWS_PER_CHUNK, w=W)
                eng = nc.gpsimd if k >= 5 else nc.vector
                eng.tensor_tensor(pk_v[:], q_v[:],
                                  pad_view(k_pad, c, _shift_delta(k))[:], mult)
                yield
                mm(sc[:], cb16[:, 420 + k * NH * K2:420 + (k + 1) * NH * K2],
                   pk[:], start=False, stop=(k == K2 - 1))
                yield
            exp_c = smallp.tile([NH * K2, CHUNK], bf16, tag="exp")
            nc.scalar.activation(exp_c[:], sc[:], AF.Exp, bias=bqsc)  # ACT
            yield
            dn = ps_s.tile([NH * K2, CHUNK], f32, tag="pss")
            mm(dn[:], rsw, exp_c[:], start=True, stop=True)
            yield
            rcp = smallp.tile([NH * K2, CHUNK], f32, tag="rcp")
            nc.vector.reciprocal_approx_fast(rcp[:], dn[:])
            yield
            rcp_bf = smallp.tile([NH * K2, CHUNK], bf16, tag="rcpbf")
            nc.scalar.copy(rcp_bf[:], rcp[:])                       # ACT
            yield
            attn_c = smallp.tile([NH * K2, CHUNK], bf16, tag="attn")
            nc.vector.tensor_tensor(attn_c[:], exp_c[:], rcp_bf[:], mult)
            attn_tiles[c] = attn_c
            yield

        def av_gen(c, split=False):
            sl = slice(c * CHUNK, (c + 1) * CHUNK)
            attn_c = attn_tiles[c]
            acc = ps_o.tile([COUT, CHUNK], f32, tag="acc")
            acc2 = None
            if split:
                acc2 = ps_a.tile([COUT, CHUNK], f32, tag="ps_a")
            # rep matmuls write bf16 PSUM (exact: 0/1 matrix x bf16 attn);
            # mk multiplies read PSUM directly as 2-byte packed operands.
            # 'e': ACT-evict + DVE bf16 mult for a couple of ks to offload DVE.
            modes = ['e', 'd', 'e', 'd', 'e', 'd', 'e', 'd', 'e']
            for k in range(K2):
                rep = ps_r.tile([CIN, CHUNK], f32, tag="rep")
                mm(rep[:], cb16[0:NH * K2, 873 + k * CIN:873 + (k + 1) * CIN],
                   attn_c[:], start=True, stop=True)
                yield
                mk = mkp.tile([CIN, CHUNK], bf16, tag="mk")
                mk_v = mk[:].rearrange("p (r w) -> p r w", r=ROWS_PER_CHUNK, w=W)
                vv = pad_view(v_pad, c, _shift_delta(k))
                if modes[k] == 'd':
                    nc.vector.tensor_tensor(
                        mk_v[:], rep[:].rearrange("p (r w) -> p r w",
                                                  r=ROWS_PER_CHUNK, w=W),
                        vv[:], mult)
                    yield
                else:
                    rep_sb = repp.tile([CIN, CHUNK], bf16, tag="repsb")
                    nc.scalar.copy(rep_sb[:], rep[:])
                    yield
                    nc.vector.tensor_tensor(
                        mk_v[:], rep_sb[:].rearrange("p (r w) -> p r w",
                                                     r=ROWS_PER_CHUNK, w=W),
                        vv[:], mult)
                    yield
                if split and k % 2 == 1:
                    mm(acc2[:], wf, mk[:], start=(k == 1), stop=(k == K2 - 2))
                else:
                    mm(acc[:], wf, mk[:], start=(k == 0), stop=(k == K2 - 1))
                yield
            out_sb = outp.tile([COUT, CHUNK], f32, tag="outsb")
            nc.scalar.add(out_sb[:], acc[:], bfb)                # ACT
            if split:
                yield
                nc.vector.tensor_tensor(out_sb[:], out_sb[:], acc2[:],
                                        mybir.AluOpType.add)
            yield
            nc.sync.dma_start(out_ext[:, sl], out_sb[:])
            yield

        def run_all(gens):
            gens = [g for g in gens if g is not None]
            while gens:
                alive = []
                for g in gens:
                    try:
                        next(g)
                        alive.append(g)
                    except StopIteration:
                        pass
                gens = alive

        attn_tiles = [None] * NCHUNK
        for qr in range(4):
            run_all([stats_quarter(qr)])
        run_all([proj_gen(0)])
        run_all([proj_gen(1)])
        run_all([scores_gen(0), proj_gen(2)])
        # steady 2-deep software pipeline: SCORES(c) | AV(c-1) | PROJ(c+2)
        for c in range(1, NCHUNK):
            run_all([scores_gen(c), av_gen(c - 1),
                     proj_gen(c + 2) if c + 2 < NCHUNK else None])
        run_all([av_gen(NCHUNK - 1, split=True)])


def _get_compiled():
    if "nc" not in _CACHE:
        _CACHE["nc"] = _build_bass()
    return _CACHE["nc"]


def kernel(**inputs):
    x = np.asarray(inputs["x"], dtype=np.float32)          # [B, CIN, H, W]
    consts = _host_fold(
        np.asarray(inputs["ln_g"]), np.asarray(inputs["ln_b"]),
        np.asarray(inputs["Wq"]), np.asarray(inputs["bq"]),
        np.asarray(inputs["Wk"]), np.asarray(inputs["bk"]),
        np.asarray(inputs["Wv"]), np.asarray(inputs["bv"]),
        np.asarray(inputs["Wp"]), np.asarray(inputs["bp"]),
        np.asarray(inputs["Wf"]), np.asarray(inputs["bf"]),
    )

    nc = _get_compiled()

    from concourse.bass_utils import run_bass_kernel_spmd

    core_ids = list(range(B))
    in_maps = []
    for i in range(B):
        m = {"x": np.ascontiguousarray(x[i].reshape(CIN, NPIX))}
        m.update(consts)
        in_maps.append(m)

    res = run_bass_kernel_spmd(nc, in_maps, core_ids,
                               trace=bool(int(os.environ.get("KTRACE", "0"))))
    _CACHE["last_result"] = res
    out = np.stack([res.results[i]["out"].reshape(COUT, H, W) for i in range(B)])
    return out.astype(np.float32)


if __name__ == "__main__":
    nc = _get_compiled()
    print("compiled OK")
